# revision 27
# baseline (speedup 1.0000x reference)
"""Trainium2 Bass kernel for nn_Attention_30227979829300.

Multi-head attention (b=4, n=2048, dim=1024, 16 heads x 64) with
interleaved-pair RoPE + Fourier positional encoding, sharded
tensor-parallel by heads across 8 NeuronCores (2 heads per core).

v2: all matmuls in bf16 (fp32r costs ~2 PE cycles/row on silicon, bf16
costs 1), rotate_half computed on-chip with one signed-permutation
matmul per 512-token block instead of two extra full projection units,
exp() emits bf16 directly for the attn@V matmul.

Per-core plan (layouts transposed so softmax needs no on-chip
transposes and no max-subtraction):
  - qkv projection: q^T/k^T/v^T in [head_dim, tokens] bf16, PSUM f32
  - rot(q)/rot(k) via block-diag signed permutation matmul
  - RoPE: q_rope = q*cos + rot(q)*sin + fenc, DVE/Pool elementwise
  - scores s^T[j, i] = sum_d k[j,d] q[i,d]  (j on partitions)
  - p = exp(s/8) on ACT straight out of PSUM -> bf16 (softmax
    denominator deferred; no max subtraction needed at these magnitudes)
  - out^T[d, i] = sum_j v[j, d] p[j, i], with a fused ones column in the
    stationary operand producing the denominator row for free
  - normalize via fast-reciprocal + GpSimd partition broadcast
  - out-projection with out^T chunks stationary -> token-major partial
    [tokens, 1024] written to DRAM
Host sums the 8 partials (the tensor-parallel all-reduce) and adds b_out.
"""

import sys

if "/opt/trn_rl_repo" not in sys.path:
    sys.path.insert(0, "/opt/trn_rl_repo")

import numpy as np
import ml_dtypes

import concourse.bass as bass
import concourse.tile as tile
from concourse import bacc, mybir
from concourse.bass_utils import run_bass_kernel_spmd

F32 = mybir.dt.float32
BF16 = mybir.dt.bfloat16
ACT_EXP = mybir.ActivationFunctionType.Exp
NP_BF16 = ml_dtypes.bfloat16

B, N, DIM = 4, 2048, 1024
HEADS, DH = 16, 64
INNER = HEADS * DH
NF = 16  # fourier freqs
T = B * N  # 8192 flat tokens
NCORES = 8
SCALE = DH ** -0.5


def _build_program():
    nc = bacc.Bacc("TRN2", target_bir_lowering=False, debug=False,
                   num_devices=NCORES)

    d = lambda name, shape, dt, kind: nc.dram_tensor(name, shape, dt, kind=kind).ap()
    xT = d("xT", [DIM, T], BF16, "ExternalInput")
    wq = d("wq", [DIM, 128], BF16, "ExternalInput")
    wk = d("wk", [DIM, 128], BF16, "ExternalInput")
    wv = d("wv", [DIM, 128], BF16, "ExternalInput")
    rotmT = d("rotmT", [128, 128], BF16, "ExternalInput")
    wo = d("wo", [128, DIM], BF16, "ExternalInput")
    cos2 = d("cos2", [128, N], BF16, "ExternalInput")
    sin2 = d("sin2", [128, N], BF16, "ExternalInput")
    fourT = d("fourT", [2 * NF, N], BF16, "ExternalInput")
    wfT = d("wfT", [2 * NF, DH], BF16, "ExternalInput")
    bf = d("bf", [DH, 1], F32, "ExternalInput")
    onesv = d("onesv", [128, 32], BF16, "ExternalInput")
    out = d("out", [T, DIM], F32, "ExternalOutput")

    with tile.TileContext(nc) as tc:
        with tc.tile_pool(name="consts", bufs=1) as consts, \
             tc.tile_pool(name="xt", bufs=32) as xtp, \
             tc.tile_pool(name="qk", bufs=2) as qkp, \
             tc.tile_pool(name="vsb", bufs=2) as vsbp, \
             tc.tile_pool(name="sbc", bufs=2) as sbcp, \
             tc.tile_pool(name="vtmp", bufs=2) as vtmpp, \
             tc.tile_pool(name="vtt", bufs=4) as vttp, \
             tc.tile_pool(name="ptil", bufs=18) as ptilp, \
             tc.tile_pool(name="ropet", bufs=2) as ropetp, \
             tc.tile_pool(name="outT", bufs=8) as outTp, \
             tc.tile_pool(name="ostg", bufs=4) as ostgp, \
             tc.tile_pool(name="arow", bufs=2) as arowp, \
             tc.tile_pool(name="acc", bufs=2, space="PSUM") as accp, \
             tc.tile_pool(name="pacc", bufs=2, space="PSUM") as paccp, \
             tc.tile_pool(name="avacc", bufs=1, space="PSUM") as avaccp, \
             tc.tile_pool(name="small", bufs=1, space="PSUM") as smallp:

            # ---- load constants ----
            # DMA issue order is latency-critical at startup: the Sync queue
            # drains in order, so tiny tensors needed by the first PE ops
            # (fenc matmul, first proj block) go first.
            four_sb = consts.tile([2 * NF, N], BF16, tag="four")
            nc.sync.dma_start(four_sb[:], fourT[:])
            wf_sb = consts.tile([2 * NF, DH], BF16, tag="wf")
            nc.sync.dma_start(wf_sb[:], wfT[:])
            bf_sb = consts.tile([DH, 1], F32, tag="bf")
            nc.sync.dma_start(bf_sb[:], bf[:])

            w_sb = {}

            def load_w(name, ap):
                t = consts.tile([128, 8 * 128], BF16, tag=name)
                nc.sync.dma_start(
                    t[:].rearrange("p (c d) -> p c d", c=8),
                    ap.rearrange("(c p) d -> p c d", p=128))
                w_sb[name] = t

            load_w("wq", wq)
            # prefetch the first projection block's xT tiles ahead of the
            # big constant DMAs so PE can start as early as possible
            xts0 = []
            for fc in range(8):
                xt_t = xtp.tile([128, 512], BF16, tag="xt")
                nc.sync.dma_start(xt_t[:], xT[bass.ts(fc, 128), 0:512])
                xts0.append(xt_t)
            onesv_sb = consts.tile([128, 32], BF16, tag="onesv")
            nc.sync.dma_start(onesv_sb[:], onesv[:])
            load_w("wk", wk)
            load_w("wv", wv)
            cos_sb = consts.tile([128, N], BF16, tag="cos")
            nc.sync.dma_start(cos_sb[:], cos2[:])
            sin_sb = consts.tile([128, N], BF16, tag="sin")
            nc.sync.dma_start(sin_sb[:], sin2[:])
            rotm_sb = consts.tile([128, 128], BF16, tag="rotm")
            nc.sync.dma_start(rotm_sb[:], rotmT[:])
            wo_sb = consts.tile([128, DIM], BF16, tag="wo")
            nc.sync.dma_start(wo_sb[:], wo[:])

            # ---- fenc2 [128, 2048]: fourier @ w_fproj.T + b_fproj, duplicated per head ----
            fenc_sb = consts.tile([128, N], BF16, tag="fenc")
            for blk in range(4):
                fp = smallp.tile([DH, 512], F32, tag="small")
                nc.tensor.matmul(fp[:], wf_sb[:], four_sb[:, bass.ts(blk, 512)],
                                 start=True, stop=True)
                nc.scalar.add(fenc_sb[0:64, bass.ts(blk, 512)], fp[:], bf_sb[:])
                nc.scalar.add(fenc_sb[64:128, bass.ts(blk, 512)], fp[:], bf_sb[:])

            batch_tiles = {}
            xts_pre = {(0, 0): xts0}

            def prefetch_xt(b, blk):
                """Issue the 8 input-tile DMAs for proj block (b, blk); called
                one attention unit ahead so the Sync queue's in-order issue
                never leaves the PE waiting on input data."""
                tok0 = b * N
                xts = []
                for fc in range(8):
                    xt_t = xtp.tile([128, 512], BF16, tag="xt")
                    nc.sync.dma_start(
                        xt_t[:],
                        xT[bass.ts(fc, 128), tok0 + blk * 512:tok0 + (blk + 1) * 512])
                    xts.append(xt_t)
                xts_pre[(b, blk)] = xts

            def proj_block_gen(b, blk):
                """Project q/k/v for 512 tokens of batch b, apply RoPE (with
                on-chip rotate_half permutation matmuls), transpose v to
                natural layout.  Yields between small chunks of PE work so
                the driver can interleave it into the ACT-paced attention
                stream (bf16 matmuls are short; without filler the PE idles
                behind exp() and HAM throttles the clock)."""
                if blk == 0:
                    q_rope = qkp.tile([128, N], BF16, tag="q")
                    k_rope = qkp.tile([128, N], BF16, tag="k")
                    v_sb = vsbp.tile([128, 16 * 130], BF16, tag="v")
                    # ones columns (col 64 of each [65]-block, both heads)
                    nc.vector.tensor_copy(
                        bass.AP(tensor=v_sb[:].tensor, offset=v_sb[:].offset + 64,
                                ap=[v_sb[:].ap[0], [130, 16], [65, 2]]),
                        onesv_sb[:].rearrange("p (a t) -> p a t", t=2))
                    batch_tiles[b] = (q_rope, k_rope, v_sb)
                q_rope, k_rope, v_sb = batch_tiles[b]
                if (b, blk) in xts_pre:
                    xts = xts_pre.pop((b, blk))
                else:
                    prefetch_xt(b, blk)
                    xts = xts_pre.pop((b, blk))
                yield
                pu = {}
                sbt = {}
                for u, stag in (("wq", "qsb"), ("wk", "ksb"), ("wv", "vt")):
                    p = paccp.tile([128, 512], F32, tag="pacc")
                    for fc in range(8):
                        nc.tensor.matmul(p[:], w_sb[u][:, bass.ts(fc, 128)],
                                         xts[fc][:],
                                         start=(fc == 0), stop=(fc == 7))
                        if fc % 2 == 1:
                            yield
                    # PSUM -> SBUF bf16 staging (DVE; GpSimd has no PSUM port)
                    pool = vtmpp if u == "wv" else sbcp
                    s = pool.tile([128, 512], BF16, tag=stag)
                    nc.vector.tensor_copy(s[:], p[:])
                    pu[u] = p
                    sbt[u] = s
                    yield
                q_sb, k_sb, vt = sbt["wq"], sbt["wk"], sbt["wv"]
                # rotate_half via signed permutation matmul (PSUM reuses pacc ring)
                qr = paccp.tile([128, 512], F32, tag="pacc")
                nc.tensor.matmul(qr[:], rotm_sb[:], q_sb[:], start=True, stop=True)
                yield
                kr = paccp.tile([128, 512], F32, tag="pacc")
                nc.tensor.matmul(kr[:], rotm_sb[:], k_sb[:], start=True, stop=True)
                yield
                bsl = bass.ts(blk, 512)
                for src, rot, dst in ((q_sb, qr, q_rope), (k_sb, kr, k_rope)):
                    t1 = ropetp.tile([128, 512], BF16, tag="t1")
                    nc.vector.tensor_mul(t1[:], src[:], cos_sb[:, bsl])
                    t2 = ropetp.tile([128, 512], BF16, tag="t2")
                    nc.vector.tensor_mul(t2[:], rot[:], sin_sb[:, bsl])
                    yield
                    t3 = ropetp.tile([128, 512], BF16, tag="t3")
                    nc.vector.tensor_add(t3[:], t1[:], t2[:])
                    nc.vector.tensor_add(dst[:, bsl], t3[:], fenc_sb[:, bsl])
                    yield
                for tt in range(4):
                    jc = blk * 4 + tt
                    # transpose via the DMA XBAR instead of the PE: no PSUM
                    # tile, no tensor-engine work
                    vtt = vttp.tile([128, 128], BF16, tag="vtt")
                    nc.sync.dma_start(vtt[:], vt[:, bass.ts(tt, 128)],
                                      transpose=True)
                    # both 64-col head halves in one strided copy, skipping
                    # the ones column at +64
                    nc.vector.tensor_copy(
                        bass.AP(tensor=v_sb[:].tensor,
                                offset=v_sb[:].offset + jc * 130,
                                ap=[v_sb[:].ap[0], [65, 2], [1, 64]]),
                        vtt[:].rearrange("p (a t) -> p a t", a=2))
                    yield

            def outproj_gen(b, ib, ot):
                tok0 = b * N
                for ic in range(4):
                    for oc in range(2):
                        po = smallp.tile([128, 512], F32, tag="small")
                        nc.tensor.matmul(po[:], ot[:, bass.ts(ic, 128)],
                                         wo_sb[:, bass.ts(oc, 512)],
                                         start=True, stop=True)
                        og = ostgp.tile([128, 512], F32, tag="og")
                        # alternate the PSUM->SBUF staging between DVE and
                        # ACT so neither queue backs up ahead of
                        # latency-critical ops
                        if oc == 0:
                            nc.vector.tensor_copy(og[:], po[:])
                        else:
                            nc.scalar.copy(og[:], po[:])
                        r0 = tok0 + ib * 512 + ic * 128
                        nc.sync.dma_start(
                            out[r0:r0 + 128, bass.ts(oc, 512)], og[:])
                        yield

            def attn_tail(op_, ot, hp):
                ar0 = arowp.tile([1, 512], F32, tag="ar0")
                nc.vector.tensor_copy(ar0[:], op_[64:65, :])
                ar = arowp.tile([1, 512], F32, tag="ar")
                nc.vector.reciprocal_approx_fast(ar[:], ar0[:])
                bc = arowp.tile([64, 512], F32, tag="bc")
                nc.gpsimd.partition_broadcast(bc[:], ar[:])
                nc.vector.tensor_mul(ot[hp, :], op_[0:64, :], bc[:])

            def attn_pair(b, pb, h, ot0, ot1, drive):
                """Scores (F=1024, both i-halves at once) + exp + attn@V for
                one head over a 1024-token i-pair.  Phase A (i-half 0)
                consumes exp output as it lands; phase B replays the resident
                pt tiles with no ACT dependency, giving the PE a wait-free
                stretch.  drive(n) pulls filler (proj/outproj) steps."""
                q_rope, k_rope, v_sb = batch_tiles[b]
                hp = slice(h * 64, (h + 1) * 64)
                q_mv = q_rope[hp, pb * 1024:(pb + 1) * 1024]
                pts = [None] * 16

                def emit_av(op_, jc, half):
                    nc.tensor.matmul(
                        op_[:],
                        v_sb[:, jc * 130 + h * 65:jc * 130 + h * 65 + 65],
                        pts[jc][:, bass.ts(half, 512)],
                        start=(jc == 0), stop=(jc == 15))

                op_a = avaccp.tile([65, 512], F32, tag="av")
                for jc in range(16):
                    sg = accp.tile([128, 1024], F32, tag="acc")
                    # one matmul per i-half: a single f32 matmul output may
                    # not cross a 2KB PSUM bank (max 512 f32 columns)
                    for ih in range(2):
                        nc.tensor.matmul(sg[:, bass.ts(ih, 512)],
                                         k_rope[hp, bass.ts(jc, 128)],
                                         q_mv[:, bass.ts(ih, 512)],
                                         start=True, stop=True)
                    pt = ptilp.tile([128, 1024], BF16, tag="pt")
                    nc.scalar.activation(pt[:], sg[:], ACT_EXP, scale=SCALE)
                    pts[jc] = pt
                    if jc >= 2:
                        emit_av(op_a, jc - 2, 0)
                    drive(2)
                emit_av(op_a, 14, 0)
                emit_av(op_a, 15, 0)
                attn_tail(op_a, ot0, hp)
                op_b = avaccp.tile([65, 512], F32, tag="av")
                for jc in range(16):
                    emit_av(op_b, jc, 1)
                    if jc % 4 == 3:
                        drive(1)
                attn_tail(op_b, ot1, hp)

            # Static startup: batch 0's four blocks (attention contracts over
            # ALL 2048 keys, so a batch's projection must fully precede its
            # first attention pair — filler stays one whole batch ahead).
            for blk in range(4):
                for _ in proj_block_gen(0, blk):
                    pass
            prefetch_xt(1, 0)
            prefetch_xt(1, 1)
            # All out-projections are deferred at least one attention pair so
            # their matmuls always have a fully-written ot tile and act as PE
            # filler; batches 1-2 hold extra back so batch 3's pairs (which
            # have less projection work to interleave) stay PE-dense.
            pending = []
            pairs = [(b, pb) for b in range(B) for pb in range(2)]
            for p, (b, pb) in enumerate(pairs):
                # prefetch inputs for the blocks filled NEXT pair
                if p + 1 < len(pairs):
                    nb, npb = pairs[p + 1]
                    if nb + 1 < B:
                        prefetch_xt(nb + 1, 2 * npb)
                        prefetch_xt(nb + 1, 2 * npb + 1)
                fill = []
                if b + 1 < B:
                    fill.append(proj_block_gen(b + 1, 2 * pb))
                    fill.append(proj_block_gen(b + 1, 2 * pb + 1))
                npop = (2 if b == 0 else
                        (min(2, max(0, len(pending) - 4)) if b < 3 else 4))
                for _ in range(npop):
                    if pending:
                        fill.append(outproj_gen(*pending.pop(0)))

                def drive(n, fill=fill):
                    for _ in range(n):
                        while fill:
                            try:
                                next(fill[0])
                                break
                            except StopIteration:
                                fill.pop(0)

                ot0 = outTp.tile([128, 512], BF16, tag="ot")
                ot1 = outTp.tile([128, 512], BF16, tag="ot")
                attn_pair(b, pb, 0, ot0, ot1, drive)
                attn_pair(b, pb, 1, ot0, ot1, drive)
                drive(99)
                pending.append((b, 2 * pb, ot0))
                pending.append((b, 2 * pb + 1, ot1))
            for args in pending:
                for _ in outproj_gen(*args):
                    pass

    nc.compile()
    return nc


_NC = None


def _get_nc():
    global _NC
    if _NC is None:
        _NC = _build_program()
    return _NC


def _host_prep(x, w_qkv, w_fproj, b_fproj, w_out, b_out):
    bt = lambda a: np.ascontiguousarray(np.asarray(a, dtype=np.float32),
                                        dtype=np.float32).astype(NP_BF16)
    xT = bt(x.reshape(T, DIM).T)

    pos = np.arange(N, dtype=np.float64)[:, None]
    freqs = 10000.0 ** (-np.arange(0, DH, 2, dtype=np.float64) / DH)
    ang = pos * freqs
    sin = np.repeat(np.sin(ang), 2, axis=1)  # [N, 64] interleave-dup
    cos = np.repeat(np.cos(ang), 2, axis=1)
    cos2 = np.tile(cos.T, (2, 1)).astype(NP_BF16)
    sin2 = np.tile(sin.T, (2, 1)).astype(NP_BF16)
    ff = np.arange(1, NF + 1, dtype=np.float64)
    fourier = np.concatenate([np.sin(pos * ff), np.cos(pos * ff)], axis=1)
    fourT = fourier.T.astype(NP_BF16)
    wfT = bt(w_fproj.T)
    bf = np.ascontiguousarray(b_fproj[:, None], dtype=np.float32)
    onesv = np.ones((128, 32), dtype=NP_BF16)

    # rotate_half as a signed permutation: rot(q)[d] = sign[d] * q[perm[d]]
    # lhsT layout for matmul: rotmT[src, dst] = sign[dst] where src=perm[dst]
    perm = np.empty(DH, np.int64)
    sign = np.empty(DH, np.float32)
    perm[:32] = 2 * np.arange(32) + 1
    sign[:32] = -1.0
    perm[32:] = 2 * np.arange(32)
    sign[32:] = 1.0
    rotmT = np.zeros((128, 128), dtype=NP_BF16)
    for hb in range(2):
        for dl in range(DH):
            rotmT[hb * DH + perm[dl], hb * DH + dl] = sign[dl]

    in_maps = []
    for c in range(NCORES):
        rows = np.concatenate([np.arange(h * DH, (h + 1) * DH)
                               for h in (2 * c, 2 * c + 1)])
        Wq = w_qkv[rows]
        Wk = w_qkv[INNER + rows]
        Wv = w_qkv[2 * INNER + rows]

        in_maps.append({
            "xT": xT,
            "wq": bt(Wq.T), "wk": bt(Wk.T), "wv": bt(Wv.T),
            "rotmT": rotmT,
            "wo": bt(w_out[:, rows].T),
            "cos2": cos2, "sin2": sin2,
            "fourT": fourT, "wfT": wfT, "bf": bf,
            "onesv": onesv,
        })
    return in_maps


LAST_RESULT = None


def kernel(x, w_qkv, w_fproj, b_fproj, w_out, b_out, *, trace=False):
    global LAST_RESULT
    x = np.asarray(x, dtype=np.float32)
    w_qkv = np.asarray(w_qkv, dtype=np.float32)
    w_fproj = np.asarray(w_fproj, dtype=np.float32)
    b_fproj = np.asarray(b_fproj, dtype=np.float32)
    w_out = np.asarray(w_out, dtype=np.float32)
    b_out = np.asarray(b_out, dtype=np.float32)

    nc = _get_nc()
    in_maps = _host_prep(x, w_qkv, w_fproj, b_fproj, w_out, b_out)
    res = run_bass_kernel_spmd(nc, in_maps, core_ids=list(range(NCORES)),
                               trace=trace)
    LAST_RESULT = res
    acc = np.zeros((T, DIM), dtype=np.float64)
    for c in range(NCORES):
        acc += res.results[c]["out"]
    acc += b_out
    return acc.reshape(B, N, DIM).astype(np.float32)


# revision 28
# speedup vs baseline: 1.1247x; 1.1247x over previous
"""Trainium2 Bass kernel for nn_Attention_30227979829300.

Multi-head attention (b=4, n=2048, dim=1024, 16 heads x 64) with
interleaved-pair RoPE + Fourier positional encoding, sharded
tensor-parallel by heads across 8 NeuronCores (2 heads per core).

v2: all matmuls in bf16 (fp32r costs ~2 PE cycles/row on silicon, bf16
costs 1), rotate_half computed on-chip with one signed-permutation
matmul per 512-token block instead of two extra full projection units,
exp() emits bf16 directly for the attn@V matmul.

Per-core plan (layouts transposed so softmax needs no on-chip
transposes and no max-subtraction):
  - qkv projection: q^T/k^T/v^T in [head_dim, tokens] bf16, PSUM f32
  - rot(q)/rot(k) via block-diag signed permutation matmul
  - RoPE: q_rope = q*cos + rot(q)*sin + fenc, DVE/Pool elementwise
  - scores s^T[j, i] = sum_d k[j,d] q[i,d]  (j on partitions)
  - p = exp(s/8) on ACT straight out of PSUM -> bf16 (softmax
    denominator deferred; no max subtraction needed at these magnitudes)
  - out^T[d, i] = sum_j v[j, d] p[j, i], with a fused ones column in the
    stationary operand producing the denominator row for free
  - normalize via fast-reciprocal + GpSimd partition broadcast
  - out-projection with out^T chunks stationary -> token-major partial
    [tokens, 1024] written to DRAM
Host sums the 8 partials (the tensor-parallel all-reduce) and adds b_out.
"""

import sys

if "/opt/trn_rl_repo" not in sys.path:
    sys.path.insert(0, "/opt/trn_rl_repo")

import numpy as np
import ml_dtypes

import concourse.bass as bass
import concourse.tile as tile
from concourse import bacc, mybir
from concourse.bass_utils import run_bass_kernel_spmd

F32 = mybir.dt.float32
BF16 = mybir.dt.bfloat16
ACT_EXP = mybir.ActivationFunctionType.Exp
NP_BF16 = ml_dtypes.bfloat16

B, N, DIM = 4, 2048, 1024
HEADS, DH = 16, 64
INNER = HEADS * DH
NF = 16  # fourier freqs
T = B * N  # 8192 flat tokens
NCORES = 8
SCALE = DH ** -0.5


def _build_program():
    nc = bacc.Bacc("TRN2", target_bir_lowering=False, debug=False,
                   num_devices=NCORES)

    d = lambda name, shape, dt, kind: nc.dram_tensor(name, shape, dt, kind=kind).ap()
    xT = d("xT", [DIM, T], BF16, "ExternalInput")
    wq = d("wq", [DIM, 128], BF16, "ExternalInput")
    wk = d("wk", [DIM, 128], BF16, "ExternalInput")
    wv = d("wv", [DIM, 128], BF16, "ExternalInput")
    rotmT = d("rotmT", [128, 128], BF16, "ExternalInput")
    wo = d("wo", [128, DIM], BF16, "ExternalInput")
    cos2 = d("cos2", [128, N], BF16, "ExternalInput")
    sin2 = d("sin2", [128, N], BF16, "ExternalInput")
    fourT = d("fourT", [2 * NF, N], BF16, "ExternalInput")
    wfT = d("wfT", [2 * NF, DH], BF16, "ExternalInput")
    bf = d("bf", [DH, 1], F32, "ExternalInput")
    ident = d("ident", [128, 128], BF16, "ExternalInput")
    onesv = d("onesv", [128, 32], BF16, "ExternalInput")
    out = d("out", [T, DIM], F32, "ExternalOutput")

    with tile.TileContext(nc) as tc:
        with tc.tile_pool(name="consts", bufs=1) as consts, \
             tc.tile_pool(name="xt", bufs=32) as xtp, \
             tc.tile_pool(name="qk", bufs=2) as qkp, \
             tc.tile_pool(name="vsb", bufs=2) as vsbp, \
             tc.tile_pool(name="sbc", bufs=2) as sbcp, \
             tc.tile_pool(name="vtmp", bufs=2) as vtmpp, \
             tc.tile_pool(name="ptil", bufs=18) as ptilp, \
             tc.tile_pool(name="ropet", bufs=2) as ropetp, \
             tc.tile_pool(name="outT", bufs=8) as outTp, \
             tc.tile_pool(name="ostg", bufs=4) as ostgp, \
             tc.tile_pool(name="arow", bufs=2) as arowp, \
             tc.tile_pool(name="acc", bufs=2, space="PSUM") as accp, \
             tc.tile_pool(name="pacc", bufs=2, space="PSUM") as paccp, \
             tc.tile_pool(name="avacc", bufs=1, space="PSUM") as avaccp, \
             tc.tile_pool(name="small", bufs=1, space="PSUM") as smallp:

            # ---- load constants ----
            # DMA issue order is latency-critical at startup: the Sync queue
            # drains in order, so tiny tensors needed by the first PE ops
            # (fenc matmul, first proj block) go first.
            four_sb = consts.tile([2 * NF, N], BF16, tag="four")
            nc.sync.dma_start(four_sb[:], fourT[:])
            wf_sb = consts.tile([2 * NF, DH], BF16, tag="wf")
            nc.sync.dma_start(wf_sb[:], wfT[:])
            bf_sb = consts.tile([DH, 1], F32, tag="bf")
            nc.sync.dma_start(bf_sb[:], bf[:])

            w_sb = {}

            def load_w(name, ap):
                t = consts.tile([128, 8 * 128], BF16, tag=name)
                nc.sync.dma_start(
                    t[:].rearrange("p (c d) -> p c d", c=8),
                    ap.rearrange("(c p) d -> p c d", p=128))
                w_sb[name] = t

            load_w("wq", wq)
            # prefetch the first projection block's xT tiles ahead of the
            # big constant DMAs so PE can start as early as possible
            xts0 = []
            for fc in range(8):
                xt_t = xtp.tile([128, 512], BF16, tag="xt")
                nc.sync.dma_start(xt_t[:], xT[bass.ts(fc, 128), 0:512])
                xts0.append(xt_t)
            onesv_sb = consts.tile([128, 32], BF16, tag="onesv")
            nc.sync.dma_start(onesv_sb[:], onesv[:])
            load_w("wk", wk)
            load_w("wv", wv)
            cos_sb = consts.tile([128, N], BF16, tag="cos")
            nc.sync.dma_start(cos_sb[:], cos2[:])
            sin_sb = consts.tile([128, N], BF16, tag="sin")
            nc.sync.dma_start(sin_sb[:], sin2[:])
            rotm_sb = consts.tile([128, 128], BF16, tag="rotm")
            nc.sync.dma_start(rotm_sb[:], rotmT[:])
            id_sb = consts.tile([128, 128], BF16, tag="ident")
            nc.sync.dma_start(id_sb[:], ident[:])
            wo_sb = consts.tile([128, DIM], BF16, tag="wo")
            nc.sync.dma_start(wo_sb[:], wo[:])

            # ---- fenc2 [128, 2048]: fourier @ w_fproj.T + b_fproj, duplicated per head ----
            fenc_sb = consts.tile([128, N], BF16, tag="fenc")
            for blk in range(4):
                fp = smallp.tile([DH, 512], F32, tag="small")
                nc.tensor.matmul(fp[:], wf_sb[:], four_sb[:, bass.ts(blk, 512)],
                                 start=True, stop=True)
                nc.scalar.add(fenc_sb[0:64, bass.ts(blk, 512)], fp[:], bf_sb[:])
                nc.scalar.add(fenc_sb[64:128, bass.ts(blk, 512)], fp[:], bf_sb[:])

            batch_tiles = {}
            xts_pre = {(0, 0): xts0}

            def prefetch_xt(b, blk):
                """Issue the 8 input-tile DMAs for proj block (b, blk); called
                one attention unit ahead so the Sync queue's in-order issue
                never leaves the PE waiting on input data."""
                tok0 = b * N
                xts = []
                for fc in range(8):
                    xt_t = xtp.tile([128, 512], BF16, tag="xt")
                    nc.sync.dma_start(
                        xt_t[:],
                        xT[bass.ts(fc, 128), tok0 + blk * 512:tok0 + (blk + 1) * 512])
                    xts.append(xt_t)
                xts_pre[(b, blk)] = xts

            def proj_block_gen(b, blk):
                """Project q/k/v for 512 tokens of batch b, apply RoPE (with
                on-chip rotate_half permutation matmuls), transpose v to
                natural layout.  Yields between small chunks of PE work so
                the driver can interleave it into the ACT-paced attention
                stream (bf16 matmuls are short; without filler the PE idles
                behind exp() and HAM throttles the clock)."""
                if blk == 0:
                    q_rope = qkp.tile([128, N], BF16, tag="q")
                    k_rope = qkp.tile([128, N], BF16, tag="k")
                    v_sb = vsbp.tile([128, 16 * 130], BF16, tag="v")
                    # ones columns (col 64 of each [65]-block, both heads)
                    nc.vector.tensor_copy(
                        bass.AP(tensor=v_sb[:].tensor, offset=v_sb[:].offset + 64,
                                ap=[v_sb[:].ap[0], [130, 16], [65, 2]]),
                        onesv_sb[:].rearrange("p (a t) -> p a t", t=2))
                    batch_tiles[b] = (q_rope, k_rope, v_sb)
                q_rope, k_rope, v_sb = batch_tiles[b]
                if (b, blk) in xts_pre:
                    xts = xts_pre.pop((b, blk))
                else:
                    prefetch_xt(b, blk)
                    xts = xts_pre.pop((b, blk))
                yield
                pu = {}
                sbt = {}
                for u, stag in (("wq", "qsb"), ("wk", "ksb"), ("wv", "vt")):
                    p = paccp.tile([128, 512], F32, tag="pacc")
                    for fc in range(8):
                        nc.tensor.matmul(p[:], w_sb[u][:, bass.ts(fc, 128)],
                                         xts[fc][:],
                                         start=(fc == 0), stop=(fc == 7))
                        if fc % 2 == 1:
                            yield
                    # PSUM -> SBUF bf16 staging (DVE; GpSimd has no PSUM port)
                    pool = vtmpp if u == "wv" else sbcp
                    s = pool.tile([128, 512], BF16, tag=stag)
                    nc.vector.tensor_copy(s[:], p[:])
                    pu[u] = p
                    sbt[u] = s
                    yield
                q_sb, k_sb, vt = sbt["wq"], sbt["wk"], sbt["wv"]
                # rotate_half via signed permutation matmul (PSUM reuses pacc ring)
                qr = paccp.tile([128, 512], F32, tag="pacc")
                nc.tensor.matmul(qr[:], rotm_sb[:], q_sb[:], start=True, stop=True)
                yield
                kr = paccp.tile([128, 512], F32, tag="pacc")
                nc.tensor.matmul(kr[:], rotm_sb[:], k_sb[:], start=True, stop=True)
                yield
                bsl = bass.ts(blk, 512)
                for src, rot, dst in ((q_sb, qr, q_rope), (k_sb, kr, k_rope)):
                    t1 = ropetp.tile([128, 512], BF16, tag="t1")
                    nc.vector.tensor_mul(t1[:], src[:], cos_sb[:, bsl])
                    t2 = ropetp.tile([128, 512], BF16, tag="t2")
                    nc.vector.tensor_mul(t2[:], rot[:], sin_sb[:, bsl])
                    yield
                    t3 = ropetp.tile([128, 512], BF16, tag="t3")
                    nc.vector.tensor_add(t3[:], t1[:], t2[:])
                    nc.vector.tensor_add(dst[:, bsl], t3[:], fenc_sb[:, bsl])
                    yield
                for tt in range(4):
                    jc = blk * 4 + tt
                    ptp = smallp.tile([128, 128], BF16, tag="small")
                    nc.tensor.transpose(ptp[:], vt[:, bass.ts(tt, 128)], id_sb[:])
                    # both 64-col head halves in one strided copy, skipping
                    # the ones column at +64
                    nc.vector.tensor_copy(
                        bass.AP(tensor=v_sb[:].tensor,
                                offset=v_sb[:].offset + jc * 130,
                                ap=[v_sb[:].ap[0], [65, 2], [1, 64]]),
                        ptp[:].rearrange("p (a t) -> p a t", a=2))
                    yield

            def outproj_gen(b, ib, ot):
                tok0 = b * N
                for ic in range(4):
                    for oc in range(2):
                        po = smallp.tile([128, 512], F32, tag="small")
                        nc.tensor.matmul(po[:], ot[:, bass.ts(ic, 128)],
                                         wo_sb[:, bass.ts(oc, 512)],
                                         start=True, stop=True)
                        og = ostgp.tile([128, 512], F32, tag="og")
                        nc.vector.tensor_copy(og[:], po[:])
                        r0 = tok0 + ib * 512 + ic * 128
                        # output DMAs dispatch from the GpSimd queue so they
                        # never head-of-line block input fetches on Sync
                        nc.gpsimd.dma_start(
                            out[r0:r0 + 128, bass.ts(oc, 512)], og[:])
                        yield

            def attn_tail(op_, ot, hp):
                ar0 = arowp.tile([1, 512], F32, tag="ar0")
                nc.vector.tensor_copy(ar0[:], op_[64:65, :])
                ar = arowp.tile([1, 512], F32, tag="ar")
                nc.vector.reciprocal_approx_fast(ar[:], ar0[:])
                bc = arowp.tile([64, 512], F32, tag="bc")
                nc.gpsimd.partition_broadcast(bc[:], ar[:])
                nc.vector.tensor_mul(ot[hp, :], op_[0:64, :], bc[:])

            def attn_pair(b, pb, h, ot0, ot1, drive):
                """Scores (F=1024, both i-halves at once) + exp + attn@V for
                one head over a 1024-token i-pair.  Phase A (i-half 0)
                consumes exp output as it lands; phase B replays the resident
                pt tiles with no ACT dependency, giving the PE a wait-free
                stretch.  drive(n) pulls filler (proj/outproj) steps."""
                q_rope, k_rope, v_sb = batch_tiles[b]
                hp = slice(h * 64, (h + 1) * 64)
                q_mv = q_rope[hp, pb * 1024:(pb + 1) * 1024]
                pts = [None] * 16

                def emit_av(op_, jc, half):
                    nc.tensor.matmul(
                        op_[:],
                        v_sb[:, jc * 130 + h * 65:jc * 130 + h * 65 + 65],
                        pts[jc][:, bass.ts(half, 512)],
                        start=(jc == 0), stop=(jc == 15))

                op_a = avaccp.tile([65, 512], F32, tag="av")
                for jc in range(16):
                    sg = accp.tile([128, 1024], F32, tag="acc")
                    # one matmul per i-half: a single f32 matmul output may
                    # not cross a 2KB PSUM bank (max 512 f32 columns)
                    for ih in range(2):
                        nc.tensor.matmul(sg[:, bass.ts(ih, 512)],
                                         k_rope[hp, bass.ts(jc, 128)],
                                         q_mv[:, bass.ts(ih, 512)],
                                         start=True, stop=True)
                    pt = ptilp.tile([128, 1024], BF16, tag="pt")
                    nc.scalar.activation(pt[:], sg[:], ACT_EXP, scale=SCALE)
                    pts[jc] = pt
                    if jc >= 2:
                        emit_av(op_a, jc - 2, 0)
                    drive(2)
                emit_av(op_a, 14, 0)
                emit_av(op_a, 15, 0)
                attn_tail(op_a, ot0, hp)
                op_b = avaccp.tile([65, 512], F32, tag="av")
                for jc in range(16):
                    emit_av(op_b, jc, 1)
                    if jc % 4 == 3:
                        drive(1)
                attn_tail(op_b, ot1, hp)

            # Static startup: batch 0's four blocks (attention contracts over
            # ALL 2048 keys, so a batch's projection must fully precede its
            # first attention pair — filler stays one whole batch ahead).
            for blk in range(4):
                for _ in proj_block_gen(0, blk):
                    pass
            prefetch_xt(1, 0)
            prefetch_xt(1, 1)
            # All out-projections are deferred at least one attention pair so
            # their matmuls always have a fully-written ot tile and act as PE
            # filler; batches 1-2 hold extra back so batch 3's pairs (which
            # have less projection work to interleave) stay PE-dense.
            pending = []
            pairs = [(b, pb) for b in range(B) for pb in range(2)]
            for p, (b, pb) in enumerate(pairs):
                # prefetch inputs for the blocks filled NEXT pair
                if p + 1 < len(pairs):
                    nb, npb = pairs[p + 1]
                    if nb + 1 < B:
                        prefetch_xt(nb + 1, 2 * npb)
                        prefetch_xt(nb + 1, 2 * npb + 1)
                fill = []
                if b + 1 < B:
                    fill.append(proj_block_gen(b + 1, 2 * pb))
                    fill.append(proj_block_gen(b + 1, 2 * pb + 1))
                npop = (2 if b == 0 else
                        (min(2, max(0, len(pending) - 4)) if b < 3 else 4))
                for _ in range(npop):
                    if pending:
                        fill.append(outproj_gen(*pending.pop(0)))

                def drive(n, fill=fill):
                    for _ in range(n):
                        while fill:
                            try:
                                next(fill[0])
                                break
                            except StopIteration:
                                fill.pop(0)

                ot0 = outTp.tile([128, 512], BF16, tag="ot")
                ot1 = outTp.tile([128, 512], BF16, tag="ot")
                attn_pair(b, pb, 0, ot0, ot1, drive)
                attn_pair(b, pb, 1, ot0, ot1, drive)
                drive(99)
                pending.append((b, 2 * pb, ot0))
                pending.append((b, 2 * pb + 1, ot1))
            for args in pending:
                for _ in outproj_gen(*args):
                    pass

    nc.compile()
    return nc


_NC = None


def _get_nc():
    global _NC
    if _NC is None:
        _NC = _build_program()
    return _NC


def _host_prep(x, w_qkv, w_fproj, b_fproj, w_out, b_out):
    bt = lambda a: np.ascontiguousarray(np.asarray(a, dtype=np.float32),
                                        dtype=np.float32).astype(NP_BF16)
    xT = bt(x.reshape(T, DIM).T)

    pos = np.arange(N, dtype=np.float64)[:, None]
    freqs = 10000.0 ** (-np.arange(0, DH, 2, dtype=np.float64) / DH)
    ang = pos * freqs
    sin = np.repeat(np.sin(ang), 2, axis=1)  # [N, 64] interleave-dup
    cos = np.repeat(np.cos(ang), 2, axis=1)
    cos2 = np.tile(cos.T, (2, 1)).astype(NP_BF16)
    sin2 = np.tile(sin.T, (2, 1)).astype(NP_BF16)
    ff = np.arange(1, NF + 1, dtype=np.float64)
    fourier = np.concatenate([np.sin(pos * ff), np.cos(pos * ff)], axis=1)
    fourT = fourier.T.astype(NP_BF16)
    wfT = bt(w_fproj.T)
    bf = np.ascontiguousarray(b_fproj[:, None], dtype=np.float32)
    onesv = np.ones((128, 32), dtype=NP_BF16)

    # rotate_half as a signed permutation: rot(q)[d] = sign[d] * q[perm[d]]
    # lhsT layout for matmul: rotmT[src, dst] = sign[dst] where src=perm[dst]
    perm = np.empty(DH, np.int64)
    sign = np.empty(DH, np.float32)
    perm[:32] = 2 * np.arange(32) + 1
    sign[:32] = -1.0
    perm[32:] = 2 * np.arange(32)
    sign[32:] = 1.0
    identm = np.eye(128, dtype=NP_BF16)
    rotmT = np.zeros((128, 128), dtype=NP_BF16)
    for hb in range(2):
        for dl in range(DH):
            rotmT[hb * DH + perm[dl], hb * DH + dl] = sign[dl]

    in_maps = []
    for c in range(NCORES):
        rows = np.concatenate([np.arange(h * DH, (h + 1) * DH)
                               for h in (2 * c, 2 * c + 1)])
        Wq = w_qkv[rows]
        Wk = w_qkv[INNER + rows]
        Wv = w_qkv[2 * INNER + rows]

        in_maps.append({
            "xT": xT,
            "wq": bt(Wq.T), "wk": bt(Wk.T), "wv": bt(Wv.T),
            "rotmT": rotmT,
            "wo": bt(w_out[:, rows].T),
            "cos2": cos2, "sin2": sin2,
            "fourT": fourT, "wfT": wfT, "bf": bf, "ident": identm,
            "onesv": onesv,
        })
    return in_maps


LAST_RESULT = None


def kernel(x, w_qkv, w_fproj, b_fproj, w_out, b_out, *, trace=False):
    global LAST_RESULT
    x = np.asarray(x, dtype=np.float32)
    w_qkv = np.asarray(w_qkv, dtype=np.float32)
    w_fproj = np.asarray(w_fproj, dtype=np.float32)
    b_fproj = np.asarray(b_fproj, dtype=np.float32)
    w_out = np.asarray(w_out, dtype=np.float32)
    b_out = np.asarray(b_out, dtype=np.float32)

    nc = _get_nc()
    in_maps = _host_prep(x, w_qkv, w_fproj, b_fproj, w_out, b_out)
    res = run_bass_kernel_spmd(nc, in_maps, core_ids=list(range(NCORES)),
                               trace=trace)
    LAST_RESULT = res
    acc = np.zeros((T, DIM), dtype=np.float64)
    for c in range(NCORES):
        acc += res.results[c]["out"]
    acc += b_out
    return acc.reshape(B, N, DIM).astype(np.float32)


# revision 29
# speedup vs baseline: 1.1319x; 1.0064x over previous
"""Trainium2 Bass kernel for nn_Attention_30227979829300.

Multi-head attention (b=4, n=2048, dim=1024, 16 heads x 64) with
interleaved-pair RoPE + Fourier positional encoding, sharded
tensor-parallel by heads across 8 NeuronCores (2 heads per core).

v2: all matmuls in bf16 (fp32r costs ~2 PE cycles/row on silicon, bf16
costs 1), rotate_half computed on-chip with one signed-permutation
matmul per 512-token block instead of two extra full projection units,
exp() emits bf16 directly for the attn@V matmul.

Per-core plan (layouts transposed so softmax needs no on-chip
transposes and no max-subtraction):
  - qkv projection: q^T/k^T/v^T in [head_dim, tokens] bf16, PSUM f32
  - rot(q)/rot(k) via block-diag signed permutation matmul
  - RoPE: q_rope = q*cos + rot(q)*sin + fenc, DVE/Pool elementwise
  - scores s^T[j, i] = sum_d k[j,d] q[i,d]  (j on partitions)
  - p = exp(s/8) on ACT straight out of PSUM -> bf16 (softmax
    denominator deferred; no max subtraction needed at these magnitudes)
  - out^T[d, i] = sum_j v[j, d] p[j, i], with a fused ones column in the
    stationary operand producing the denominator row for free
  - normalize via fast-reciprocal + GpSimd partition broadcast
  - out-projection with out^T chunks stationary -> token-major partial
    [tokens, 1024] written to DRAM
Host sums the 8 partials (the tensor-parallel all-reduce) and adds b_out.
"""

import sys

if "/opt/trn_rl_repo" not in sys.path:
    sys.path.insert(0, "/opt/trn_rl_repo")

import numpy as np
import ml_dtypes

import concourse.bass as bass
import concourse.tile as tile
from concourse import bacc, mybir
from concourse.bass_utils import run_bass_kernel_spmd

F32 = mybir.dt.float32
BF16 = mybir.dt.bfloat16
ACT_EXP = mybir.ActivationFunctionType.Exp
NP_BF16 = ml_dtypes.bfloat16

B, N, DIM = 4, 2048, 1024
HEADS, DH = 16, 64
INNER = HEADS * DH
NF = 16  # fourier freqs
T = B * N  # 8192 flat tokens
NCORES = 8
SCALE = DH ** -0.5


def _build_program():
    nc = bacc.Bacc("TRN2", target_bir_lowering=False, debug=False,
                   num_devices=NCORES)

    d = lambda name, shape, dt, kind: nc.dram_tensor(name, shape, dt, kind=kind).ap()
    xT = d("xT", [DIM, T], BF16, "ExternalInput")
    wq = d("wq", [DIM, 128], BF16, "ExternalInput")
    wk = d("wk", [DIM, 128], BF16, "ExternalInput")
    wv = d("wv", [DIM, 128], BF16, "ExternalInput")
    rotmT = d("rotmT", [128, 128], BF16, "ExternalInput")
    wo = d("wo", [128, DIM], BF16, "ExternalInput")
    cos2 = d("cos2", [128, N], BF16, "ExternalInput")
    sin2 = d("sin2", [128, N], BF16, "ExternalInput")
    fourT = d("fourT", [2 * NF, N], BF16, "ExternalInput")
    wfT = d("wfT", [2 * NF, DH], BF16, "ExternalInput")
    bf = d("bf", [DH, 1], F32, "ExternalInput")
    ident = d("ident", [128, 128], BF16, "ExternalInput")
    onesv = d("onesv", [128, 32], BF16, "ExternalInput")
    out = d("out", [T, DIM], F32, "ExternalOutput")

    with tile.TileContext(nc) as tc:
        with tc.tile_pool(name="consts", bufs=1) as consts, \
             tc.tile_pool(name="xt", bufs=32) as xtp, \
             tc.tile_pool(name="qk", bufs=2) as qkp, \
             tc.tile_pool(name="vsb", bufs=2) as vsbp, \
             tc.tile_pool(name="sbc", bufs=2) as sbcp, \
             tc.tile_pool(name="vtmp", bufs=2) as vtmpp, \
             tc.tile_pool(name="ptil", bufs=18) as ptilp, \
             tc.tile_pool(name="ropet", bufs=2) as ropetp, \
             tc.tile_pool(name="outT", bufs=8) as outTp, \
             tc.tile_pool(name="ostg", bufs=4) as ostgp, \
             tc.tile_pool(name="arow", bufs=2) as arowp, \
             tc.tile_pool(name="acc", bufs=2, space="PSUM") as accp, \
             tc.tile_pool(name="pacc", bufs=2, space="PSUM") as paccp, \
             tc.tile_pool(name="avacc", bufs=1, space="PSUM") as avaccp, \
             tc.tile_pool(name="small", bufs=1, space="PSUM") as smallp:

            # ---- load constants ----
            # DMA issue order is latency-critical at startup: the Sync queue
            # drains in order, so tiny tensors needed by the first PE ops
            # (fenc matmul, first proj block) go first.
            four_sb = consts.tile([2 * NF, N], BF16, tag="four")
            nc.sync.dma_start(four_sb[:], fourT[:])
            wf_sb = consts.tile([2 * NF, DH], BF16, tag="wf")
            nc.sync.dma_start(wf_sb[:], wfT[:])
            bf_sb = consts.tile([DH, 1], F32, tag="bf")
            nc.sync.dma_start(bf_sb[:], bf[:])

            w_sb = {}

            def load_w(name, ap):
                t = consts.tile([128, 8 * 128], BF16, tag=name)
                nc.sync.dma_start(
                    t[:].rearrange("p (c d) -> p c d", c=8),
                    ap.rearrange("(c p) d -> p c d", p=128))
                w_sb[name] = t

            load_w("wq", wq)
            # prefetch the first projection block's xT tiles ahead of the
            # big constant DMAs so PE can start as early as possible
            xts0 = []
            for fc in range(8):
                xt_t = xtp.tile([128, 512], BF16, tag="xt")
                nc.sync.dma_start(xt_t[:], xT[bass.ts(fc, 128), 0:512])
                xts0.append(xt_t)
            onesv_sb = consts.tile([128, 32], BF16, tag="onesv")
            nc.sync.dma_start(onesv_sb[:], onesv[:])
            load_w("wk", wk)
            load_w("wv", wv)
            cos_sb = consts.tile([128, N], BF16, tag="cos")
            nc.sync.dma_start(cos_sb[:], cos2[:])
            sin_sb = consts.tile([128, N], BF16, tag="sin")
            nc.sync.dma_start(sin_sb[:], sin2[:])
            rotm_sb = consts.tile([128, 128], BF16, tag="rotm")
            nc.sync.dma_start(rotm_sb[:], rotmT[:])
            id_sb = consts.tile([128, 128], BF16, tag="ident")
            nc.sync.dma_start(id_sb[:], ident[:])
            wo_sb = consts.tile([128, DIM], BF16, tag="wo")
            nc.sync.dma_start(wo_sb[:], wo[:])

            # ---- fenc2 [128, 2048]: fourier @ w_fproj.T + b_fproj, duplicated per head ----
            fenc_sb = consts.tile([128, N], BF16, tag="fenc")
            for blk in range(4):
                fp = smallp.tile([DH, 512], F32, tag="small")
                nc.tensor.matmul(fp[:], wf_sb[:], four_sb[:, bass.ts(blk, 512)],
                                 start=True, stop=True)
                nc.scalar.add(fenc_sb[0:64, bass.ts(blk, 512)], fp[:], bf_sb[:])
                nc.scalar.add(fenc_sb[64:128, bass.ts(blk, 512)], fp[:], bf_sb[:])

            batch_tiles = {}
            xts_pre = {(0, 0): xts0}

            def prefetch_xt(b, blk):
                """Issue the 8 input-tile DMAs for proj block (b, blk); called
                one attention unit ahead so the Sync queue's in-order issue
                never leaves the PE waiting on input data."""
                tok0 = b * N
                xts = []
                for fc in range(8):
                    xt_t = xtp.tile([128, 512], BF16, tag="xt")
                    nc.sync.dma_start(
                        xt_t[:],
                        xT[bass.ts(fc, 128), tok0 + blk * 512:tok0 + (blk + 1) * 512])
                    xts.append(xt_t)
                xts_pre[(b, blk)] = xts

            def proj_block_gen(b, blk):
                """Project q/k/v for 512 tokens of batch b, apply RoPE (with
                on-chip rotate_half permutation matmuls), transpose v to
                natural layout.  Yields between small chunks of PE work so
                the driver can interleave it into the ACT-paced attention
                stream (bf16 matmuls are short; without filler the PE idles
                behind exp() and HAM throttles the clock)."""
                if blk == 0:
                    q_rope = qkp.tile([128, N], BF16, tag="q")
                    k_rope = qkp.tile([128, N], BF16, tag="k")
                    v_sb = vsbp.tile([128, 16 * 130], BF16, tag="v")
                    # ones columns (col 64 of each [65]-block, both heads)
                    nc.vector.tensor_copy(
                        bass.AP(tensor=v_sb[:].tensor, offset=v_sb[:].offset + 64,
                                ap=[v_sb[:].ap[0], [130, 16], [65, 2]]),
                        onesv_sb[:].rearrange("p (a t) -> p a t", t=2))
                    batch_tiles[b] = (q_rope, k_rope, v_sb)
                q_rope, k_rope, v_sb = batch_tiles[b]
                if (b, blk) in xts_pre:
                    xts = xts_pre.pop((b, blk))
                else:
                    prefetch_xt(b, blk)
                    xts = xts_pre.pop((b, blk))
                yield
                pu = {}
                sbt = {}
                for u, stag in (("wq", "qsb"), ("wk", "ksb"), ("wv", "vt")):
                    p = paccp.tile([128, 512], F32, tag="pacc")
                    for fc in range(8):
                        nc.tensor.matmul(p[:], w_sb[u][:, bass.ts(fc, 128)],
                                         xts[fc][:],
                                         start=(fc == 0), stop=(fc == 7))
                        if fc % 2 == 1:
                            yield
                    # PSUM -> SBUF bf16 staging (DVE; GpSimd has no PSUM port)
                    pool = vtmpp if u == "wv" else sbcp
                    s = pool.tile([128, 512], BF16, tag=stag)
                    nc.vector.tensor_copy(s[:], p[:])
                    pu[u] = p
                    sbt[u] = s
                    yield
                q_sb, k_sb, vt = sbt["wq"], sbt["wk"], sbt["wv"]
                # rotate_half via signed permutation matmul (PSUM reuses pacc ring)
                qr = paccp.tile([128, 512], F32, tag="pacc")
                nc.tensor.matmul(qr[:], rotm_sb[:], q_sb[:], start=True, stop=True)
                yield
                kr = paccp.tile([128, 512], F32, tag="pacc")
                nc.tensor.matmul(kr[:], rotm_sb[:], k_sb[:], start=True, stop=True)
                yield
                bsl = bass.ts(blk, 512)
                for src, rot, dst in ((q_sb, qr, q_rope), (k_sb, kr, k_rope)):
                    t1 = ropetp.tile([128, 512], BF16, tag="t1")
                    nc.vector.tensor_mul(t1[:], src[:], cos_sb[:, bsl])
                    t2 = ropetp.tile([128, 512], BF16, tag="t2")
                    nc.vector.tensor_mul(t2[:], rot[:], sin_sb[:, bsl])
                    yield
                    t3 = ropetp.tile([128, 512], BF16, tag="t3")
                    nc.vector.tensor_add(t3[:], t1[:], t2[:])
                    nc.vector.tensor_add(dst[:, bsl], t3[:], fenc_sb[:, bsl])
                    yield
                for tt in range(4):
                    jc = blk * 4 + tt
                    ptp = smallp.tile([128, 128], BF16, tag="small")
                    nc.tensor.transpose(ptp[:], vt[:, bass.ts(tt, 128)], id_sb[:])
                    # both 64-col head halves in one strided copy, skipping
                    # the ones column at +64
                    nc.vector.tensor_copy(
                        bass.AP(tensor=v_sb[:].tensor,
                                offset=v_sb[:].offset + jc * 130,
                                ap=[v_sb[:].ap[0], [65, 2], [1, 64]]),
                        ptp[:].rearrange("p (a t) -> p a t", a=2))
                    yield

            def outproj_gen(b, ib, ot):
                tok0 = b * N
                for ic in range(4):
                    for oc in range(2):
                        po = smallp.tile([128, 512], F32, tag="small")
                        nc.tensor.matmul(po[:], ot[:, bass.ts(ic, 128)],
                                         wo_sb[:, bass.ts(oc, 512)],
                                         start=True, stop=True)
                        og = ostgp.tile([128, 512], F32, tag="og")
                        nc.vector.tensor_copy(og[:], po[:])
                        r0 = tok0 + ib * 512 + ic * 128
                        # HWDGE issue on Sync is ~600ns vs ~770ns SWDGE on
                        # GpSimd; inputs are prefetched a whole pair ahead so
                        # head-of-line blocking behind og is harmless
                        nc.sync.dma_start(
                            out[r0:r0 + 128, bass.ts(oc, 512)], og[:])
                        yield

            def attn_tail(op_, ot, hp):
                # ACT sits idle right at tail time (between h-phases), so the
                # denominator-row staging copy clears instantly there instead
                # of queueing behind DVE work
                ar0 = arowp.tile([1, 512], F32, tag="ar0")
                nc.scalar.copy(ar0[:], op_[64:65, :])
                ar = arowp.tile([1, 512], F32, tag="ar")
                nc.vector.reciprocal_approx_fast(ar[:], ar0[:])
                bc = arowp.tile([64, 512], F32, tag="bc")
                nc.gpsimd.partition_broadcast(bc[:], ar[:])
                nc.vector.tensor_mul(ot[hp, :], op_[0:64, :], bc[:])

            def attn_pair(b, pb, h, ot0, ot1, drive):
                """Scores (F=1024, both i-halves at once) + exp + attn@V for
                one head over a 1024-token i-pair.  Phase A (i-half 0)
                consumes exp output as it lands; phase B replays the resident
                pt tiles with no ACT dependency, giving the PE a wait-free
                stretch.  drive(n) pulls filler (proj/outproj) steps."""
                q_rope, k_rope, v_sb = batch_tiles[b]
                hp = slice(h * 64, (h + 1) * 64)
                q_mv = q_rope[hp, pb * 1024:(pb + 1) * 1024]
                pts = [None] * 16

                def emit_av(op_, jc, half):
                    nc.tensor.matmul(
                        op_[:],
                        v_sb[:, jc * 130 + h * 65:jc * 130 + h * 65 + 65],
                        pts[jc][:, bass.ts(half, 512)],
                        start=(jc == 0), stop=(jc == 15))

                op_a = avaccp.tile([65, 512], F32, tag="av")
                for jc in range(16):
                    sg = accp.tile([128, 1024], F32, tag="acc")
                    # one matmul per i-half: a single f32 matmul output may
                    # not cross a 2KB PSUM bank (max 512 f32 columns)
                    for ih in range(2):
                        nc.tensor.matmul(sg[:, bass.ts(ih, 512)],
                                         k_rope[hp, bass.ts(jc, 128)],
                                         q_mv[:, bass.ts(ih, 512)],
                                         start=True, stop=True)
                    pt = ptilp.tile([128, 1024], BF16, tag="pt")
                    nc.scalar.activation(pt[:], sg[:], ACT_EXP, scale=SCALE)
                    pts[jc] = pt
                    if jc >= 3:
                        emit_av(op_a, jc - 3, 0)
                    drive(2)
                for jc in (13, 14, 15):
                    emit_av(op_a, jc, 0)
                attn_tail(op_a, ot0, hp)
                # burst filler while the tail chain drains on DVE/GpSimd so
                # the PE isn't parked on op_b's bank-reuse wait
                drive(5)
                op_b = avaccp.tile([65, 512], F32, tag="av")
                for jc in range(16):
                    emit_av(op_b, jc, 1)
                    if jc % 4 == 3:
                        drive(1)
                attn_tail(op_b, ot1, hp)

            # Static startup: batch 0's four blocks (attention contracts over
            # ALL 2048 keys, so a batch's projection must fully precede its
            # first attention pair — filler stays one whole batch ahead).
            for blk in range(4):
                for _ in proj_block_gen(0, blk):
                    pass
            prefetch_xt(1, 0)
            prefetch_xt(1, 1)
            # All out-projections are deferred at least one attention pair so
            # their matmuls always have a fully-written ot tile and act as PE
            # filler; batches 1-2 hold extra back so batch 3's pairs (which
            # have less projection work to interleave) stay PE-dense.
            pending = []
            pairs = [(b, pb) for b in range(B) for pb in range(2)]
            for p, (b, pb) in enumerate(pairs):
                # prefetch inputs for the blocks filled NEXT pair
                if p + 1 < len(pairs):
                    nb, npb = pairs[p + 1]
                    if nb + 1 < B:
                        prefetch_xt(nb + 1, 2 * npb)
                        prefetch_xt(nb + 1, 2 * npb + 1)
                fill = []
                if b + 1 < B:
                    fill.append(proj_block_gen(b + 1, 2 * pb))
                    fill.append(proj_block_gen(b + 1, 2 * pb + 1))
                npop = (2 if b == 0 else
                        (min(2, max(0, len(pending) - 4)) if b < 3 else 4))
                for _ in range(npop):
                    if pending:
                        fill.append(outproj_gen(*pending.pop(0)))

                def drive(n, fill=fill):
                    for _ in range(n):
                        while fill:
                            try:
                                next(fill[0])
                                break
                            except StopIteration:
                                fill.pop(0)

                ot0 = outTp.tile([128, 512], BF16, tag="ot")
                ot1 = outTp.tile([128, 512], BF16, tag="ot")
                attn_pair(b, pb, 0, ot0, ot1, drive)
                attn_pair(b, pb, 1, ot0, ot1, drive)
                drive(99)
                pending.append((b, 2 * pb, ot0))
                pending.append((b, 2 * pb + 1, ot1))
            for args in pending:
                for _ in outproj_gen(*args):
                    pass

    nc.compile()
    return nc


_NC = None


def _get_nc():
    global _NC
    if _NC is None:
        _NC = _build_program()
    return _NC


def _host_prep(x, w_qkv, w_fproj, b_fproj, w_out, b_out):
    bt = lambda a: np.ascontiguousarray(np.asarray(a, dtype=np.float32),
                                        dtype=np.float32).astype(NP_BF16)
    xT = bt(x.reshape(T, DIM).T)

    pos = np.arange(N, dtype=np.float64)[:, None]
    freqs = 10000.0 ** (-np.arange(0, DH, 2, dtype=np.float64) / DH)
    ang = pos * freqs
    sin = np.repeat(np.sin(ang), 2, axis=1)  # [N, 64] interleave-dup
    cos = np.repeat(np.cos(ang), 2, axis=1)
    cos2 = np.tile(cos.T, (2, 1)).astype(NP_BF16)
    sin2 = np.tile(sin.T, (2, 1)).astype(NP_BF16)
    ff = np.arange(1, NF + 1, dtype=np.float64)
    fourier = np.concatenate([np.sin(pos * ff), np.cos(pos * ff)], axis=1)
    fourT = fourier.T.astype(NP_BF16)
    wfT = bt(w_fproj.T)
    bf = np.ascontiguousarray(b_fproj[:, None], dtype=np.float32)
    onesv = np.ones((128, 32), dtype=NP_BF16)

    # rotate_half as a signed permutation: rot(q)[d] = sign[d] * q[perm[d]]
    # lhsT layout for matmul: rotmT[src, dst] = sign[dst] where src=perm[dst]
    perm = np.empty(DH, np.int64)
    sign = np.empty(DH, np.float32)
    perm[:32] = 2 * np.arange(32) + 1
    sign[:32] = -1.0
    perm[32:] = 2 * np.arange(32)
    sign[32:] = 1.0
    identm = np.eye(128, dtype=NP_BF16)
    rotmT = np.zeros((128, 128), dtype=NP_BF16)
    for hb in range(2):
        for dl in range(DH):
            rotmT[hb * DH + perm[dl], hb * DH + dl] = sign[dl]

    in_maps = []
    for c in range(NCORES):
        rows = np.concatenate([np.arange(h * DH, (h + 1) * DH)
                               for h in (2 * c, 2 * c + 1)])
        Wq = w_qkv[rows]
        Wk = w_qkv[INNER + rows]
        Wv = w_qkv[2 * INNER + rows]

        in_maps.append({
            "xT": xT,
            "wq": bt(Wq.T), "wk": bt(Wk.T), "wv": bt(Wv.T),
            "rotmT": rotmT,
            "wo": bt(w_out[:, rows].T),
            "cos2": cos2, "sin2": sin2,
            "fourT": fourT, "wfT": wfT, "bf": bf, "ident": identm,
            "onesv": onesv,
        })
    return in_maps


LAST_RESULT = None


def kernel(x, w_qkv, w_fproj, b_fproj, w_out, b_out, *, trace=False):
    global LAST_RESULT
    x = np.asarray(x, dtype=np.float32)
    w_qkv = np.asarray(w_qkv, dtype=np.float32)
    w_fproj = np.asarray(w_fproj, dtype=np.float32)
    b_fproj = np.asarray(b_fproj, dtype=np.float32)
    w_out = np.asarray(w_out, dtype=np.float32)
    b_out = np.asarray(b_out, dtype=np.float32)

    nc = _get_nc()
    in_maps = _host_prep(x, w_qkv, w_fproj, b_fproj, w_out, b_out)
    res = run_bass_kernel_spmd(nc, in_maps, core_ids=list(range(NCORES)),
                               trace=trace)
    LAST_RESULT = res
    acc = np.zeros((T, DIM), dtype=np.float64)
    for c in range(NCORES):
        acc += res.results[c]["out"]
    acc += b_out
    return acc.reshape(B, N, DIM).astype(np.float32)


# revision 31
# speedup vs baseline: 1.1376x; 1.0050x over previous
"""Trainium2 Bass kernel for nn_Attention_30227979829300.

Multi-head attention (b=4, n=2048, dim=1024, 16 heads x 64) with
interleaved-pair RoPE + Fourier positional encoding, sharded
tensor-parallel by heads across 8 NeuronCores (2 heads per core).

v2: all matmuls in bf16 (fp32r costs ~2 PE cycles/row on silicon, bf16
costs 1), rotate_half computed on-chip with one signed-permutation
matmul per 512-token block instead of two extra full projection units,
exp() emits bf16 directly for the attn@V matmul.

Per-core plan (layouts transposed so softmax needs no on-chip
transposes and no max-subtraction):
  - qkv projection: q^T/k^T/v^T in [head_dim, tokens] bf16, PSUM f32
  - rot(q)/rot(k) via block-diag signed permutation matmul
  - RoPE: q_rope = q*cos + rot(q)*sin + fenc, DVE/Pool elementwise
  - scores s^T[j, i] = sum_d k[j,d] q[i,d]  (j on partitions)
  - p = exp(s/8) on ACT straight out of PSUM -> bf16 (softmax
    denominator deferred; no max subtraction needed at these magnitudes)
  - out^T[d, i] = sum_j v[j, d] p[j, i], with a fused ones column in the
    stationary operand producing the denominator row for free
  - normalize via fast-reciprocal + GpSimd partition broadcast
  - out-projection with out^T chunks stationary -> token-major partial
    [tokens, 1024] written to DRAM
Host sums the 8 partials (the tensor-parallel all-reduce) and adds b_out.
"""

import sys

if "/opt/trn_rl_repo" not in sys.path:
    sys.path.insert(0, "/opt/trn_rl_repo")

import numpy as np
import ml_dtypes

import concourse.bass as bass
import concourse.tile as tile
from concourse import bacc, mybir
from concourse.bass_utils import run_bass_kernel_spmd

F32 = mybir.dt.float32
BF16 = mybir.dt.bfloat16
ACT_EXP = mybir.ActivationFunctionType.Exp
NP_BF16 = ml_dtypes.bfloat16

B, N, DIM = 4, 2048, 1024
HEADS, DH = 16, 64
INNER = HEADS * DH
NF = 16  # fourier freqs
T = B * N  # 8192 flat tokens
NCORES = 8
SCALE = DH ** -0.5


def _build_program():
    nc = bacc.Bacc("TRN2", target_bir_lowering=False, debug=False,
                   num_devices=NCORES)

    d = lambda name, shape, dt, kind: nc.dram_tensor(name, shape, dt, kind=kind).ap()
    xT = d("xT", [DIM, T], BF16, "ExternalInput")
    wq = d("wq", [DIM, 128], BF16, "ExternalInput")
    wk = d("wk", [DIM, 128], BF16, "ExternalInput")
    wv = d("wv", [DIM, 128], BF16, "ExternalInput")
    rotmT = d("rotmT", [128, 128], BF16, "ExternalInput")
    wo = d("wo", [128, DIM], BF16, "ExternalInput")
    cos2 = d("cos2", [128, N], BF16, "ExternalInput")
    sin2 = d("sin2", [128, N], BF16, "ExternalInput")
    fourT = d("fourT", [2 * NF, N], BF16, "ExternalInput")
    wfT = d("wfT", [2 * NF, DH], BF16, "ExternalInput")
    bf = d("bf", [DH, 1], F32, "ExternalInput")
    ident = d("ident", [128, 128], BF16, "ExternalInput")
    onesv = d("onesv", [128, 32], BF16, "ExternalInput")
    out = d("out", [T, DIM], F32, "ExternalOutput")

    with tile.TileContext(nc) as tc:
        with tc.tile_pool(name="consts", bufs=1) as consts, \
             tc.tile_pool(name="xt", bufs=32) as xtp, \
             tc.tile_pool(name="qk", bufs=2) as qkp, \
             tc.tile_pool(name="vsb", bufs=2) as vsbp, \
             tc.tile_pool(name="sbc", bufs=2) as sbcp, \
             tc.tile_pool(name="vtmp", bufs=2) as vtmpp, \
             tc.tile_pool(name="ptil", bufs=18) as ptilp, \
             tc.tile_pool(name="ropet", bufs=2) as ropetp, \
             tc.tile_pool(name="outT", bufs=8) as outTp, \
             tc.tile_pool(name="ostg", bufs=4) as ostgp, \
             tc.tile_pool(name="arow", bufs=2) as arowp, \
             tc.tile_pool(name="acc", bufs=2, space="PSUM") as accp, \
             tc.tile_pool(name="pacc", bufs=2, space="PSUM") as paccp, \
             tc.tile_pool(name="avacc", bufs=1, space="PSUM") as avaccp, \
             tc.tile_pool(name="small", bufs=1, space="PSUM") as smallp:

            # ---- load constants ----
            # DMA issue order is latency-critical at startup: the Sync queue
            # drains in order, so tiny tensors needed by the first PE ops
            # (fenc matmul, first proj block) go first.
            four_sb = consts.tile([2 * NF, N], BF16, tag="four")
            nc.sync.dma_start(four_sb[:], fourT[:])
            wf_sb = consts.tile([2 * NF, DH], BF16, tag="wf")
            nc.sync.dma_start(wf_sb[:], wfT[:])
            bf_sb = consts.tile([DH, 1], F32, tag="bf")
            nc.sync.dma_start(bf_sb[:], bf[:])

            w_sb = {}

            def load_w(name, ap):
                t = consts.tile([128, 8 * 128], BF16, tag=name)
                nc.sync.dma_start(
                    t[:].rearrange("p (c d) -> p c d", c=8),
                    ap.rearrange("(c p) d -> p c d", p=128))
                w_sb[name] = t

            load_w("wq", wq)
            # prefetch the first projection block's xT tiles ahead of the
            # big constant DMAs so PE can start as early as possible
            xts0 = []
            for fc in range(8):
                xt_t = xtp.tile([128, 512], BF16, tag="xt")
                nc.sync.dma_start(xt_t[:], xT[bass.ts(fc, 128), 0:512])
                xts0.append(xt_t)
            onesv_sb = consts.tile([128, 32], BF16, tag="onesv")
            nc.sync.dma_start(onesv_sb[:], onesv[:])
            load_w("wk", wk)
            load_w("wv", wv)
            cos_sb = consts.tile([128, N], BF16, tag="cos")
            nc.sync.dma_start(cos_sb[:], cos2[:])
            sin_sb = consts.tile([128, N], BF16, tag="sin")
            nc.sync.dma_start(sin_sb[:], sin2[:])
            rotm_sb = consts.tile([128, 128], BF16, tag="rotm")
            nc.sync.dma_start(rotm_sb[:], rotmT[:])
            id_sb = consts.tile([128, 128], BF16, tag="ident")
            nc.sync.dma_start(id_sb[:], ident[:])
            wo_sb = consts.tile([128, DIM], BF16, tag="wo")
            nc.sync.dma_start(wo_sb[:], wo[:])

            # ---- fenc2 [128, 2048]: fourier @ w_fproj.T + b_fproj, duplicated per head ----
            fenc_sb = consts.tile([128, N], BF16, tag="fenc")
            for blk in range(4):
                fp = smallp.tile([DH, 512], F32, tag="small")
                nc.tensor.matmul(fp[:], wf_sb[:], four_sb[:, bass.ts(blk, 512)],
                                 start=True, stop=True)
                nc.scalar.add(fenc_sb[0:64, bass.ts(blk, 512)], fp[:], bf_sb[:])
                nc.scalar.add(fenc_sb[64:128, bass.ts(blk, 512)], fp[:], bf_sb[:])

            batch_tiles = {}
            xts_pre = {(0, 0): xts0}

            def prefetch_xt(b, blk):
                """Issue the 8 input-tile DMAs for proj block (b, blk); called
                one attention unit ahead so the Sync queue's in-order issue
                never leaves the PE waiting on input data."""
                tok0 = b * N
                xts = []
                for fc in range(8):
                    xt_t = xtp.tile([128, 512], BF16, tag="xt")
                    nc.sync.dma_start(
                        xt_t[:],
                        xT[bass.ts(fc, 128), tok0 + blk * 512:tok0 + (blk + 1) * 512])
                    xts.append(xt_t)
                xts_pre[(b, blk)] = xts

            def proj_block_gen(b, blk):
                """Project q/k/v for 512 tokens of batch b, apply RoPE (with
                on-chip rotate_half permutation matmuls), transpose v to
                natural layout.  Yields between small chunks of PE work so
                the driver can interleave it into the ACT-paced attention
                stream (bf16 matmuls are short; without filler the PE idles
                behind exp() and HAM throttles the clock)."""
                if blk == 0:
                    q_rope = qkp.tile([128, N], BF16, tag="q")
                    k_rope = qkp.tile([128, N], BF16, tag="k")
                    v_sb = vsbp.tile([128, 16 * 130], BF16, tag="v")
                    # ones columns (col 64 of each [65]-block, both heads)
                    nc.vector.tensor_copy(
                        bass.AP(tensor=v_sb[:].tensor, offset=v_sb[:].offset + 64,
                                ap=[v_sb[:].ap[0], [130, 16], [65, 2]]),
                        onesv_sb[:].rearrange("p (a t) -> p a t", t=2))
                    batch_tiles[b] = (q_rope, k_rope, v_sb)
                q_rope, k_rope, v_sb = batch_tiles[b]
                if (b, blk) in xts_pre:
                    xts = xts_pre.pop((b, blk))
                else:
                    prefetch_xt(b, blk)
                    xts = xts_pre.pop((b, blk))
                yield
                pu = {}
                sbt = {}
                for u, stag in (("wq", "qsb"), ("wk", "ksb"), ("wv", "vt")):
                    p = paccp.tile([128, 512], F32, tag="pacc")
                    for fc in range(8):
                        nc.tensor.matmul(p[:], w_sb[u][:, bass.ts(fc, 128)],
                                         xts[fc][:],
                                         start=(fc == 0), stop=(fc == 7))
                        if fc % 2 == 1:
                            yield
                    # PSUM -> SBUF bf16 staging (DVE; GpSimd has no PSUM port)
                    pool = vtmpp if u == "wv" else sbcp
                    s = pool.tile([128, 512], BF16, tag=stag)
                    nc.vector.tensor_copy(s[:], p[:])
                    pu[u] = p
                    sbt[u] = s
                    yield
                q_sb, k_sb, vt = sbt["wq"], sbt["wk"], sbt["wv"]
                # rotate_half via signed permutation matmul (PSUM reuses pacc ring)
                qr = paccp.tile([128, 512], F32, tag="pacc")
                nc.tensor.matmul(qr[:], rotm_sb[:], q_sb[:], start=True, stop=True)
                yield
                kr = paccp.tile([128, 512], F32, tag="pacc")
                nc.tensor.matmul(kr[:], rotm_sb[:], k_sb[:], start=True, stop=True)
                yield
                bsl = bass.ts(blk, 512)
                for src, rot, dst in ((q_sb, qr, q_rope), (k_sb, kr, k_rope)):
                    t1 = ropetp.tile([128, 512], BF16, tag="t1")
                    nc.vector.tensor_mul(t1[:], src[:], cos_sb[:, bsl])
                    t2 = ropetp.tile([128, 512], BF16, tag="t2")
                    nc.vector.tensor_mul(t2[:], rot[:], sin_sb[:, bsl])
                    yield
                    t3 = ropetp.tile([128, 512], BF16, tag="t3")
                    nc.vector.tensor_add(t3[:], t1[:], t2[:])
                    nc.vector.tensor_add(dst[:, bsl], t3[:], fenc_sb[:, bsl])
                    yield
                for tt in range(4):
                    jc = blk * 4 + tt
                    ptp = smallp.tile([128, 128], BF16, tag="small")
                    nc.tensor.transpose(ptp[:], vt[:, bass.ts(tt, 128)], id_sb[:])
                    # both 64-col head halves in one strided copy, skipping
                    # the ones column at +64
                    nc.vector.tensor_copy(
                        bass.AP(tensor=v_sb[:].tensor,
                                offset=v_sb[:].offset + jc * 130,
                                ap=[v_sb[:].ap[0], [65, 2], [1, 64]]),
                        ptp[:].rearrange("p (a t) -> p a t", a=2))
                    yield

            def outproj_gen(b, ib, ot):
                tok0 = b * N
                for ic in range(4):
                    for oc in range(2):
                        po = smallp.tile([128, 512], F32, tag="small")
                        nc.tensor.matmul(po[:], ot[:, bass.ts(ic, 128)],
                                         wo_sb[:, bass.ts(oc, 512)],
                                         start=True, stop=True)
                        og = ostgp.tile([128, 512], F32, tag="og")
                        nc.vector.tensor_copy(og[:], po[:])
                        r0 = tok0 + ib * 512 + ic * 128
                        # HWDGE issue on Sync is ~600ns vs ~770ns SWDGE on
                        # GpSimd; inputs are prefetched a whole pair ahead so
                        # head-of-line blocking behind og is harmless
                        nc.sync.dma_start(
                            out[r0:r0 + 128, bass.ts(oc, 512)], og[:])
                        yield

            def attn_tail(op_, ot, hp):
                # ACT sits idle right at tail time (between h-phases), so the
                # denominator-row staging copy clears instantly there instead
                # of queueing behind DVE work
                ar0 = arowp.tile([1, 512], F32, tag="ar0")
                nc.scalar.copy(ar0[:], op_[64:65, :])
                ar = arowp.tile([1, 512], F32, tag="ar")
                nc.vector.reciprocal_approx_fast(ar[:], ar0[:])
                bc = arowp.tile([64, 512], F32, tag="bc")
                nc.gpsimd.partition_broadcast(bc[:], ar[:])
                nc.vector.tensor_mul(ot[hp, :], op_[0:64, :], bc[:])

            def attn_pair(b, pb, h, ot0, ot1, drive):
                """Scores (F=1024, both i-halves at once) + exp + attn@V for
                one head over a 1024-token i-pair.  Phase A (i-half 0)
                consumes exp output as it lands; phase B replays the resident
                pt tiles with no ACT dependency, giving the PE a wait-free
                stretch.  drive(n) pulls filler (proj/outproj) steps."""
                q_rope, k_rope, v_sb = batch_tiles[b]
                hp = slice(h * 64, (h + 1) * 64)
                q_mv = q_rope[hp, pb * 1024:(pb + 1) * 1024]
                pts = [None] * 16

                def emit_av(op_, jc, half):
                    nc.tensor.matmul(
                        op_[:],
                        v_sb[:, jc * 130 + h * 65:jc * 130 + h * 65 + 65],
                        pts[jc][:, bass.ts(half, 512)],
                        start=(jc == 0), stop=(jc == 15))

                op_a = avaccp.tile([65, 512], F32, tag="av")
                for jc in range(16):
                    sg = accp.tile([128, 1024], F32, tag="acc")
                    # one matmul per i-half: a single f32 matmul output may
                    # not cross a 2KB PSUM bank (max 512 f32 columns)
                    for ih in range(2):
                        nc.tensor.matmul(sg[:, bass.ts(ih, 512)],
                                         k_rope[hp, bass.ts(jc, 128)],
                                         q_mv[:, bass.ts(ih, 512)],
                                         start=True, stop=True)
                    pt = ptilp.tile([128, 1024], BF16, tag="pt")
                    nc.scalar.activation(pt[:], sg[:], ACT_EXP, scale=SCALE)
                    pts[jc] = pt
                    if jc >= 3:
                        emit_av(op_a, jc - 3, 0)
                    drive(2)
                for jc in (13, 14, 15):
                    emit_av(op_a, jc, 0)
                attn_tail(op_a, ot0, hp)
                # burst filler while the tail chain drains on DVE/GpSimd so
                # the PE isn't parked on op_b's bank-reuse wait
                drive(5)
                op_b = avaccp.tile([65, 512], F32, tag="av")
                for jc in range(16):
                    emit_av(op_b, jc, 1)
                    if jc % 4 == 3:
                        drive(1)
                attn_tail(op_b, ot1, hp)

            # Static startup: batch 0's four blocks (attention contracts over
            # ALL 2048 keys, so a batch's projection must fully precede its
            # first attention pair — filler stays one whole batch ahead).
            for blk in range(4):
                for _ in proj_block_gen(0, blk):
                    pass
            prefetch_xt(1, 0)
            prefetch_xt(1, 1)
            # All out-projections are deferred at least one attention pair so
            # their matmuls always have a fully-written ot tile and act as PE
            # filler; batches 1-2 hold extra back so batch 3's pairs (which
            # have less projection work to interleave) stay PE-dense.
            pending = []
            pairs = [(b, pb) for b in range(B) for pb in range(2)]
            for p, (b, pb) in enumerate(pairs):
                # prefetch inputs for the blocks filled NEXT pair
                if p + 1 < len(pairs):
                    nb, npb = pairs[p + 1]
                    if nb + 1 < B:
                        prefetch_xt(nb + 1, 2 * npb)
                        prefetch_xt(nb + 1, 2 * npb + 1)
                fill = []
                if b + 1 < B:
                    fill.append(proj_block_gen(b + 1, 2 * pb))
                    fill.append(proj_block_gen(b + 1, 2 * pb + 1))
                npop = (2 if b == 0 else
                        (min(2, max(0, len(pending) - 4)) if b < 3 else 4))
                for _ in range(npop):
                    if pending:
                        fill.append(outproj_gen(*pending.pop(0)))

                def drive(n, fill=fill):
                    for _ in range(n):
                        while fill:
                            try:
                                next(fill[0])
                                break
                            except StopIteration:
                                fill.pop(0)

                ot0 = outTp.tile([128, 512], BF16, tag="ot")
                ot1 = outTp.tile([128, 512], BF16, tag="ot")
                attn_pair(b, pb, 0, ot0, ot1, drive)
                attn_pair(b, pb, 1, ot0, ot1, drive)
                drive(99)
                pending.append((b, 2 * pb, ot0))
                pending.append((b, 2 * pb + 1, ot1))
            for args in pending:
                for _ in outproj_gen(*args):
                    pass

    nc.compile()
    return nc


_NC = None


def _get_nc():
    global _NC
    if _NC is None:
        _NC = _build_program()
    return _NC


def _host_prep(x, w_qkv, w_fproj, b_fproj, w_out, b_out):
    bt = lambda a: np.ascontiguousarray(np.asarray(a, dtype=np.float32),
                                        dtype=np.float32).astype(NP_BF16)
    xT = bt(x.reshape(T, DIM).T)

    pos = np.arange(N, dtype=np.float64)[:, None]
    freqs = 10000.0 ** (-np.arange(0, DH, 2, dtype=np.float64) / DH)
    ang = pos * freqs
    sin = np.repeat(np.sin(ang), 2, axis=1)  # [N, 64] interleave-dup
    cos = np.repeat(np.cos(ang), 2, axis=1)
    cos2 = np.tile(cos.T, (2, 1)).astype(NP_BF16)
    sin2 = np.tile(sin.T, (2, 1)).astype(NP_BF16)
    ff = np.arange(1, NF + 1, dtype=np.float64)
    fourier = np.concatenate([np.sin(pos * ff), np.cos(pos * ff)], axis=1)
    fourT = fourier.T.astype(NP_BF16)
    wfT = bt(w_fproj.T)
    bf = np.ascontiguousarray(b_fproj[:, None], dtype=np.float32)
    onesv = np.ones((128, 32), dtype=NP_BF16)

    # rotate_half as a signed permutation: rot(q)[d] = sign[d] * q[perm[d]]
    # lhsT layout for matmul: rotmT[src, dst] = sign[dst] where src=perm[dst]
    perm = np.empty(DH, np.int64)
    sign = np.empty(DH, np.float32)
    perm[:32] = 2 * np.arange(32) + 1
    sign[:32] = -1.0
    perm[32:] = 2 * np.arange(32)
    sign[32:] = 1.0
    identm = np.eye(128, dtype=NP_BF16)
    rotmT = np.zeros((128, 128), dtype=NP_BF16)
    for hb in range(2):
        for dl in range(DH):
            rotmT[hb * DH + perm[dl], hb * DH + dl] = sign[dl]

    in_maps = []
    for c in range(NCORES):
        rows = np.concatenate([np.arange(h * DH, (h + 1) * DH)
                               for h in (2 * c, 2 * c + 1)])
        Wq = w_qkv[rows]
        Wk = w_qkv[INNER + rows]
        Wv = w_qkv[2 * INNER + rows]

        in_maps.append({
            "xT": xT,
            "wq": bt(Wq.T), "wk": bt(Wk.T), "wv": bt(Wv.T),
            "rotmT": rotmT,
            "wo": bt(w_out[:, rows].T),
            "cos2": cos2, "sin2": sin2,
            "fourT": fourT, "wfT": wfT, "bf": bf, "ident": identm,
            "onesv": onesv,
        })
    return in_maps


LAST_RESULT = None


def kernel(x, w_qkv, w_fproj, b_fproj, w_out, b_out, *, trace=False):
    global LAST_RESULT
    x = np.asarray(x, dtype=np.float32)
    w_qkv = np.asarray(w_qkv, dtype=np.float32)
    w_fproj = np.asarray(w_fproj, dtype=np.float32)
    b_fproj = np.asarray(b_fproj, dtype=np.float32)
    w_out = np.asarray(w_out, dtype=np.float32)
    b_out = np.asarray(b_out, dtype=np.float32)

    nc = _get_nc()
    in_maps = _host_prep(x, w_qkv, w_fproj, b_fproj, w_out, b_out)
    res = run_bass_kernel_spmd(nc, in_maps, core_ids=list(range(NCORES)),
                               trace=trace)
    LAST_RESULT = res
    acc = np.zeros((T, DIM), dtype=np.float64)
    for c in range(NCORES):
        acc += res.results[c]["out"]
    acc += b_out
    return acc.reshape(B, N, DIM).astype(np.float32)


# revision 32
# speedup vs baseline: 1.1689x; 1.0275x over previous
"""Trainium2 Bass kernel for nn_Attention_30227979829300.

Multi-head attention (b=4, n=2048, dim=1024, 16 heads x 64) with
interleaved-pair RoPE + Fourier positional encoding, sharded
tensor-parallel by heads across 8 NeuronCores (2 heads per core).

v2: all matmuls in bf16 (fp32r costs ~2 PE cycles/row on silicon, bf16
costs 1), rotate_half computed on-chip with one signed-permutation
matmul per 512-token block instead of two extra full projection units,
exp() emits bf16 directly for the attn@V matmul.

Per-core plan (layouts transposed so softmax needs no on-chip
transposes and no max-subtraction):
  - qkv projection: q^T/k^T/v^T in [head_dim, tokens] bf16, PSUM f32
  - rot(q)/rot(k) via block-diag signed permutation matmul
  - RoPE: q_rope = q*cos + rot(q)*sin + fenc, DVE/Pool elementwise
  - scores s^T[j, i] = sum_d k[j,d] q[i,d]  (j on partitions)
  - p = exp(s/8) on ACT straight out of PSUM -> bf16 (softmax
    denominator deferred; no max subtraction needed at these magnitudes)
  - out^T[d, i] = sum_j v[j, d] p[j, i], with a fused ones column in the
    stationary operand producing the denominator row for free
  - normalize via fast-reciprocal + GpSimd partition broadcast
  - out-projection with out^T chunks stationary -> token-major partial
    [tokens, 1024] written to DRAM
Host sums the 8 partials (the tensor-parallel all-reduce) and adds b_out.
"""

import sys

if "/opt/trn_rl_repo" not in sys.path:
    sys.path.insert(0, "/opt/trn_rl_repo")

import numpy as np
import ml_dtypes

import concourse.bass as bass
import concourse.tile as tile
from concourse import bacc, mybir
from concourse.bass_utils import run_bass_kernel_spmd

F32 = mybir.dt.float32
BF16 = mybir.dt.bfloat16
ACT_EXP = mybir.ActivationFunctionType.Exp
NP_BF16 = ml_dtypes.bfloat16

B, N, DIM = 4, 2048, 1024
HEADS, DH = 16, 64
INNER = HEADS * DH
NF = 16  # fourier freqs
T = B * N  # 8192 flat tokens
NCORES = 8
SCALE = DH ** -0.5


def _build_program():
    nc = bacc.Bacc("TRN2", target_bir_lowering=False, debug=False,
                   num_devices=NCORES)

    d = lambda name, shape, dt, kind: nc.dram_tensor(name, shape, dt, kind=kind).ap()
    xT = d("xT", [DIM, T], BF16, "ExternalInput")
    wq = d("wq", [DIM, 128], BF16, "ExternalInput")
    wk = d("wk", [DIM, 128], BF16, "ExternalInput")
    wv = d("wv", [DIM, 128], BF16, "ExternalInput")
    rotmT = d("rotmT", [128, 128], BF16, "ExternalInput")
    wo = d("wo", [128, DIM], BF16, "ExternalInput")
    cos2 = d("cos2", [128, N], BF16, "ExternalInput")
    sin2 = d("sin2", [128, N], BF16, "ExternalInput")
    fourT = d("fourT", [2 * NF, N], BF16, "ExternalInput")
    wfT = d("wfT", [2 * NF, DH], BF16, "ExternalInput")
    bf = d("bf", [DH, 1], F32, "ExternalInput")
    ident = d("ident", [128, 128], BF16, "ExternalInput")
    onesv = d("onesv", [128, 32], BF16, "ExternalInput")
    out = d("out", [T, DIM], F32, "ExternalOutput")

    with tile.TileContext(nc) as tc:
        with tc.tile_pool(name="consts", bufs=1) as consts, \
             tc.tile_pool(name="xt", bufs=32) as xtp, \
             tc.tile_pool(name="qk", bufs=2) as qkp, \
             tc.tile_pool(name="vsb", bufs=2) as vsbp, \
             tc.tile_pool(name="sbc", bufs=2) as sbcp, \
             tc.tile_pool(name="vtmp", bufs=2) as vtmpp, \
             tc.tile_pool(name="ptil", bufs=18) as ptilp, \
             tc.tile_pool(name="ropet", bufs=2) as ropetp, \
             tc.tile_pool(name="outT", bufs=8) as outTp, \
             tc.tile_pool(name="ostg", bufs=4) as ostgp, \
             tc.tile_pool(name="arow", bufs=2) as arowp, \
             tc.tile_pool(name="acc", bufs=2, space="PSUM") as accp, \
             tc.tile_pool(name="pacc", bufs=2, space="PSUM") as paccp, \
             tc.tile_pool(name="avacc", bufs=1, space="PSUM") as avaccp, \
             tc.tile_pool(name="small", bufs=1, space="PSUM") as smallp:

            # ---- load constants ----
            # DMA issue order is latency-critical at startup: the Sync queue
            # drains in order, so tiny tensors needed by the first PE ops
            # (fenc matmul, first proj block) go first.
            four_sb = consts.tile([2 * NF, N], BF16, tag="four")
            nc.sync.dma_start(four_sb[:], fourT[:])
            wf_sb = consts.tile([2 * NF, DH], BF16, tag="wf")
            nc.sync.dma_start(wf_sb[:], wfT[:])
            bf_sb = consts.tile([DH, 1], F32, tag="bf")
            nc.sync.dma_start(bf_sb[:], bf[:])

            w_sb = {}

            def load_w(name, ap):
                t = consts.tile([128, 8 * 128], BF16, tag=name)
                nc.sync.dma_start(
                    t[:].rearrange("p (c d) -> p c d", c=8),
                    ap.rearrange("(c p) d -> p c d", p=128))
                w_sb[name] = t

            load_w("wq", wq)
            # prefetch the first projection block's xT tiles ahead of the
            # big constant DMAs so PE can start as early as possible
            xts0 = []
            for fc in range(8):
                xt_t = xtp.tile([128, 512], BF16, tag="xt")
                nc.sync.dma_start(xt_t[:], xT[bass.ts(fc, 128), 0:512])
                xts0.append(xt_t)
            onesv_sb = consts.tile([128, 32], BF16, tag="onesv")
            nc.sync.dma_start(onesv_sb[:], onesv[:])
            load_w("wk", wk)
            load_w("wv", wv)
            cos_sb = consts.tile([128, N], BF16, tag="cos")
            nc.sync.dma_start(cos_sb[:], cos2[:])
            sin_sb = consts.tile([128, N], BF16, tag="sin")
            nc.sync.dma_start(sin_sb[:], sin2[:])
            rotm_sb = consts.tile([128, 128], BF16, tag="rotm")
            nc.sync.dma_start(rotm_sb[:], rotmT[:])
            id_sb = consts.tile([128, 128], BF16, tag="ident")
            nc.sync.dma_start(id_sb[:], ident[:])
            wo_sb = consts.tile([128, DIM], BF16, tag="wo")
            nc.sync.dma_start(wo_sb[:], wo[:])

            # ---- fenc2 [128, 2048]: fourier @ w_fproj.T + b_fproj, duplicated per head ----
            fenc_sb = consts.tile([128, N], BF16, tag="fenc")
            for blk in range(4):
                fp = smallp.tile([DH, 512], F32, tag="small")
                nc.tensor.matmul(fp[:], wf_sb[:], four_sb[:, bass.ts(blk, 512)],
                                 start=True, stop=True)
                nc.scalar.add(fenc_sb[0:64, bass.ts(blk, 512)], fp[:], bf_sb[:])
                nc.scalar.add(fenc_sb[64:128, bass.ts(blk, 512)], fp[:], bf_sb[:])

            batch_tiles = {}
            xts_pre = {(0, 0): xts0}

            def prefetch_xt(b, blk):
                """Issue the 8 input-tile DMAs for proj block (b, blk); called
                one attention unit ahead so the Sync queue's in-order issue
                never leaves the PE waiting on input data."""
                tok0 = b * N
                xts = []
                for fc in range(8):
                    xt_t = xtp.tile([128, 512], BF16, tag="xt")
                    nc.sync.dma_start(
                        xt_t[:],
                        xT[bass.ts(fc, 128), tok0 + blk * 512:tok0 + (blk + 1) * 512])
                    xts.append(xt_t)
                xts_pre[(b, blk)] = xts

            def proj_block_gen(b, blk):
                """Project q/k/v for 512 tokens of batch b, apply RoPE (with
                on-chip rotate_half permutation matmuls), transpose v to
                natural layout.  Yields between small chunks of PE work so
                the driver can interleave it into the ACT-paced attention
                stream (bf16 matmuls are short; without filler the PE idles
                behind exp() and HAM throttles the clock)."""
                if blk == 0:
                    q_rope = qkp.tile([128, N], BF16, tag="q")
                    k_rope = qkp.tile([128, N], BF16, tag="k")
                    v_sb = vsbp.tile([128, 16 * 130], BF16, tag="v")
                    # ones columns (col 64 of each [65]-block, both heads)
                    nc.vector.tensor_copy(
                        bass.AP(tensor=v_sb[:].tensor, offset=v_sb[:].offset + 64,
                                ap=[v_sb[:].ap[0], [130, 16], [65, 2]]),
                        onesv_sb[:].rearrange("p (a t) -> p a t", t=2))
                    batch_tiles[b] = (q_rope, k_rope, v_sb)
                q_rope, k_rope, v_sb = batch_tiles[b]
                if (b, blk) in xts_pre:
                    xts = xts_pre.pop((b, blk))
                else:
                    prefetch_xt(b, blk)
                    xts = xts_pre.pop((b, blk))
                yield
                pu = {}
                sbt = {}
                for u, stag in (("wq", "qsb"), ("wk", "ksb"), ("wv", "vt")):
                    p = paccp.tile([128, 512], F32, tag="pacc")
                    for fc in range(8):
                        nc.tensor.matmul(p[:], w_sb[u][:, bass.ts(fc, 128)],
                                         xts[fc][:],
                                         start=(fc == 0), stop=(fc == 7))
                        if fc % 2 == 1:
                            yield
                    # PSUM -> SBUF bf16 staging (DVE; GpSimd has no PSUM port)
                    pool = vtmpp if u == "wv" else sbcp
                    s = pool.tile([128, 512], BF16, tag=stag)
                    nc.vector.tensor_copy(s[:], p[:])
                    pu[u] = p
                    sbt[u] = s
                    yield
                q_sb, k_sb, vt = sbt["wq"], sbt["wk"], sbt["wv"]
                # rotate_half via signed permutation matmul (PSUM reuses pacc ring)
                qr = paccp.tile([128, 512], F32, tag="pacc")
                nc.tensor.matmul(qr[:], rotm_sb[:], q_sb[:], start=True, stop=True)
                yield
                kr = paccp.tile([128, 512], F32, tag="pacc")
                nc.tensor.matmul(kr[:], rotm_sb[:], k_sb[:], start=True, stop=True)
                yield
                bsl = bass.ts(blk, 512)
                for src, rot, dst in ((q_sb, qr, q_rope), (k_sb, kr, k_rope)):
                    t1 = ropetp.tile([128, 512], BF16, tag="t1")
                    nc.vector.tensor_mul(t1[:], src[:], cos_sb[:, bsl])
                    t2 = ropetp.tile([128, 512], BF16, tag="t2")
                    nc.vector.tensor_mul(t2[:], rot[:], sin_sb[:, bsl])
                    yield
                    t3 = ropetp.tile([128, 512], BF16, tag="t3")
                    nc.vector.tensor_add(t3[:], t1[:], t2[:])
                    nc.vector.tensor_add(dst[:, bsl], t3[:], fenc_sb[:, bsl])
                    yield
                for tt in range(4):
                    jc = blk * 4 + tt
                    ptp = smallp.tile([128, 128], BF16, tag="small")
                    nc.tensor.transpose(ptp[:], vt[:, bass.ts(tt, 128)], id_sb[:])
                    # both 64-col head halves in one strided copy, skipping
                    # the ones column at +64
                    nc.vector.tensor_copy(
                        bass.AP(tensor=v_sb[:].tensor,
                                offset=v_sb[:].offset + jc * 130,
                                ap=[v_sb[:].ap[0], [65, 2], [1, 64]]),
                        ptp[:].rearrange("p (a t) -> p a t", a=2))
                    yield

            def outproj_gen(b, ib, ot, og_on_act=False):
                tok0 = b * N
                for ic in range(4):
                    for oc in range(2):
                        po = smallp.tile([128, 512], F32, tag="small")
                        nc.tensor.matmul(po[:], ot[:, bass.ts(ic, 128)],
                                         wo_sb[:, bass.ts(oc, 512)],
                                         start=True, stop=True)
                        og = ostgp.tile([128, 512], F32, tag="og")
                        # during the end drain ACT is idle and clears the
                        # PSUM staging instantly, so the ring-1 po bank
                        # recycles without queueing behind DVE
                        if og_on_act:
                            nc.scalar.copy(og[:], po[:])
                        else:
                            nc.vector.tensor_copy(og[:], po[:])
                        r0 = tok0 + ib * 512 + ic * 128
                        # HWDGE issue on Sync is ~600ns vs ~770ns SWDGE on
                        # GpSimd; inputs are prefetched a whole pair ahead so
                        # head-of-line blocking behind og is harmless
                        nc.sync.dma_start(
                            out[r0:r0 + 128, bass.ts(oc, 512)], og[:])
                        yield

            def attn_tail(op_, ot, hp):
                # ACT sits idle right at tail time (between h-phases), so the
                # denominator-row staging copy clears instantly there instead
                # of queueing behind DVE work
                ar0 = arowp.tile([1, 512], F32, tag="ar0")
                nc.scalar.copy(ar0[:], op_[64:65, :])
                ar = arowp.tile([1, 512], F32, tag="ar")
                nc.vector.reciprocal_approx_fast(ar[:], ar0[:])
                bc = arowp.tile([64, 512], F32, tag="bc")
                nc.gpsimd.partition_broadcast(bc[:], ar[:])
                nc.vector.tensor_mul(ot[hp, :], op_[0:64, :], bc[:])

            def attn_pair(b, pb, h, ot0, ot1, drive, tail_hook=None):
                """Scores (F=1024, both i-halves at once) + exp + attn@V for
                one head over a 1024-token i-pair.  Phase A (i-half 0)
                consumes exp output as it lands; phase B replays the resident
                pt tiles with no ACT dependency, giving the PE a wait-free
                stretch.  drive(n) pulls filler (proj/outproj) steps."""
                q_rope, k_rope, v_sb = batch_tiles[b]
                hp = slice(h * 64, (h + 1) * 64)
                q_mv = q_rope[hp, pb * 1024:(pb + 1) * 1024]
                pts = [None] * 16

                def emit_av(op_, jc, half):
                    nc.tensor.matmul(
                        op_[:],
                        v_sb[:, jc * 130 + h * 65:jc * 130 + h * 65 + 65],
                        pts[jc][:, bass.ts(half, 512)],
                        start=(jc == 0), stop=(jc == 15))

                op_a = avaccp.tile([65, 512], F32, tag="av")
                for jc in range(16):
                    sg = accp.tile([128, 1024], F32, tag="acc")
                    # one matmul per i-half: a single f32 matmul output may
                    # not cross a 2KB PSUM bank (max 512 f32 columns)
                    for ih in range(2):
                        nc.tensor.matmul(sg[:, bass.ts(ih, 512)],
                                         k_rope[hp, bass.ts(jc, 128)],
                                         q_mv[:, bass.ts(ih, 512)],
                                         start=True, stop=True)
                    pt = ptilp.tile([128, 1024], BF16, tag="pt")
                    nc.scalar.activation(pt[:], sg[:], ACT_EXP, scale=SCALE)
                    pts[jc] = pt
                    if jc >= 3:
                        emit_av(op_a, jc - 3, 0)
                    drive(2)
                for jc in (13, 14, 15):
                    emit_av(op_a, jc, 0)
                attn_tail(op_a, ot0, hp)
                if tail_hook is not None:
                    tail_hook(0)
                # burst filler while the tail chain drains on DVE/GpSimd so
                # the PE isn't parked on op_b's bank-reuse wait
                drive(5)
                op_b = avaccp.tile([65, 512], F32, tag="av")
                for jc in range(16):
                    emit_av(op_b, jc, 1)
                    if jc % 4 == 3:
                        drive(1)
                attn_tail(op_b, ot1, hp)

            # Static startup: batch 0's four blocks (attention contracts over
            # ALL 2048 keys, so a batch's projection must fully precede its
            # first attention pair — filler stays one whole batch ahead).
            for blk in range(4):
                for _ in proj_block_gen(0, blk):
                    pass
            prefetch_xt(1, 0)
            prefetch_xt(1, 1)
            # All out-projections are deferred at least one attention pair so
            # their matmuls always have a fully-written ot tile and act as PE
            # filler; batches 1-2 hold extra back so batch 3's pairs (which
            # have less projection work to interleave) stay PE-dense.
            pending = []
            pairs = [(b, pb) for b in range(B) for pb in range(2)]
            for p, (b, pb) in enumerate(pairs):
                # prefetch inputs for the blocks filled NEXT pair
                if p + 1 < len(pairs):
                    nb, npb = pairs[p + 1]
                    if nb + 1 < B:
                        prefetch_xt(nb + 1, 2 * npb)
                        prefetch_xt(nb + 1, 2 * npb + 1)
                fill = []
                if b + 1 < B:
                    fill.append(proj_block_gen(b + 1, 2 * pb))
                    fill.append(proj_block_gen(b + 1, 2 * pb + 1))
                npop = (2 if b == 0 else
                        (min(2, max(0, len(pending) - 4)) if b < 3 else 4))
                for _ in range(npop):
                    if pending:
                        fill.append(outproj_gen(*pending.pop(0)))

                def drive(n, fill=fill):
                    for _ in range(n):
                        while fill:
                            try:
                                next(fill[0])
                                break
                            except StopIteration:
                                fill.pop(0)

                ot0 = outTp.tile([128, 512], BF16, tag="ot")
                ot1 = outTp.tile([128, 512], BF16, tag="ot")
                attn_pair(b, pb, 0, ot0, ot1, drive)
                last = p == len(pairs) - 1

                def hook(half, fill=fill):
                    # last pair: overlap ot0's out-projection with h1's
                    # B-phase instead of leaving it for the serial drain
                    if half == 0:
                        fill.append(outproj_gen(b, 2 * pb, ot0,
                                                og_on_act=True))

                attn_pair(b, pb, 1, ot0, ot1, drive,
                          tail_hook=hook if last else None)
                drive(99)
                if last:
                    pending.append((b, 2 * pb + 1, ot1))
                else:
                    pending.append((b, 2 * pb, ot0))
                    pending.append((b, 2 * pb + 1, ot1))
            for args in pending:
                for _ in outproj_gen(*args, og_on_act=True):
                    pass

    nc.compile()
    return nc


_NC = None


def _get_nc():
    global _NC
    if _NC is None:
        _NC = _build_program()
    return _NC


def _host_prep(x, w_qkv, w_fproj, b_fproj, w_out, b_out):
    bt = lambda a: np.ascontiguousarray(np.asarray(a, dtype=np.float32),
                                        dtype=np.float32).astype(NP_BF16)
    xT = bt(x.reshape(T, DIM).T)

    pos = np.arange(N, dtype=np.float64)[:, None]
    freqs = 10000.0 ** (-np.arange(0, DH, 2, dtype=np.float64) / DH)
    ang = pos * freqs
    sin = np.repeat(np.sin(ang), 2, axis=1)  # [N, 64] interleave-dup
    cos = np.repeat(np.cos(ang), 2, axis=1)
    cos2 = np.tile(cos.T, (2, 1)).astype(NP_BF16)
    sin2 = np.tile(sin.T, (2, 1)).astype(NP_BF16)
    ff = np.arange(1, NF + 1, dtype=np.float64)
    fourier = np.concatenate([np.sin(pos * ff), np.cos(pos * ff)], axis=1)
    fourT = fourier.T.astype(NP_BF16)
    wfT = bt(w_fproj.T)
    bf = np.ascontiguousarray(b_fproj[:, None], dtype=np.float32)
    onesv = np.ones((128, 32), dtype=NP_BF16)

    # rotate_half as a signed permutation: rot(q)[d] = sign[d] * q[perm[d]]
    # lhsT layout for matmul: rotmT[src, dst] = sign[dst] where src=perm[dst]
    perm = np.empty(DH, np.int64)
    sign = np.empty(DH, np.float32)
    perm[:32] = 2 * np.arange(32) + 1
    sign[:32] = -1.0
    perm[32:] = 2 * np.arange(32)
    sign[32:] = 1.0
    identm = np.eye(128, dtype=NP_BF16)
    rotmT = np.zeros((128, 128), dtype=NP_BF16)
    for hb in range(2):
        for dl in range(DH):
            rotmT[hb * DH + perm[dl], hb * DH + dl] = sign[dl]

    in_maps = []
    for c in range(NCORES):
        rows = np.concatenate([np.arange(h * DH, (h + 1) * DH)
                               for h in (2 * c, 2 * c + 1)])
        Wq = w_qkv[rows]
        Wk = w_qkv[INNER + rows]
        Wv = w_qkv[2 * INNER + rows]

        in_maps.append({
            "xT": xT,
            "wq": bt(Wq.T), "wk": bt(Wk.T), "wv": bt(Wv.T),
            "rotmT": rotmT,
            "wo": bt(w_out[:, rows].T),
            "cos2": cos2, "sin2": sin2,
            "fourT": fourT, "wfT": wfT, "bf": bf, "ident": identm,
            "onesv": onesv,
        })
    return in_maps


LAST_RESULT = None


def kernel(x, w_qkv, w_fproj, b_fproj, w_out, b_out, *, trace=False):
    global LAST_RESULT
    x = np.asarray(x, dtype=np.float32)
    w_qkv = np.asarray(w_qkv, dtype=np.float32)
    w_fproj = np.asarray(w_fproj, dtype=np.float32)
    b_fproj = np.asarray(b_fproj, dtype=np.float32)
    w_out = np.asarray(w_out, dtype=np.float32)
    b_out = np.asarray(b_out, dtype=np.float32)

    nc = _get_nc()
    in_maps = _host_prep(x, w_qkv, w_fproj, b_fproj, w_out, b_out)
    res = run_bass_kernel_spmd(nc, in_maps, core_ids=list(range(NCORES)),
                               trace=trace)
    LAST_RESULT = res
    acc = np.zeros((T, DIM), dtype=np.float64)
    for c in range(NCORES):
        acc += res.results[c]["out"]
    acc += b_out
    return acc.reshape(B, N, DIM).astype(np.float32)
